# revision 1
# baseline (speedup 1.0000x reference)
"""Trainium2 Bass kernel for the deep-hedging Milstein SDE loss.

Math: the reference scan has closed-form structure. With y = [s, v]:
  s_{n+1} = s_n * m_n,  m_n = 1 + MU*dt + SIG*dW_n + 0.5*SIG^2*(dW_n^2 - dt)
  v_{n+1} = v_n + dhdt*dt + dhds*(s_{n+1}-s_n) + 0.5*SIG^2*s_n^2*dW_n^2*dhdss
where (dhdt, dhds, dhdss) are derivatives of the holding MLP h(t, s) at
(t_n, s_n).  So the scan collapses to:
  1. prefix-product along steps for s_n (tensor_tensor_scan)
  2. one fully-batched forward-mode jet evaluation of the MLP over all
     B*N points with 3 tangent streams (combined first-order gamma =
     ds-weighted + dt-weighted tangent, sqrt(c)-scaled s-tangent u, and
     c-scaled 2nd-order stream v)
  3. a per-path reduction over steps.

Layout per core (1024 paths):
  sgrid [128 part = p, 8 blocks b, 128 steps n], path_local = b*128 + p.
  MLP packs 4 groups of 32 features on partitions (group g = quartile
  g = p' block of paths); group point column j = (p'*8 + b)*128 + n.

Elementwise per hidden layer uses only the derivative_silu_and_others
ACT table set (Derivative_silu, Tanh, Square); silu itself is computed
on DVE as (z+b)*sigma with sigma = 0.5*tanh(z/2)+0.5, and
silu''(z) = sigma - silu'(z)*tanh(z/2).
"""

import numpy as np

import concourse.bass as bass
import concourse.mybir as mybir
from concourse import tile
from concourse.bass_utils import run_bass_kernel_spmd


# problem constants (hardcoded per spec)
B = 8192
NSTEP = 128
NCORE = 8
BC = B // NCORE          # 1024 paths per core
P = 128                  # partitions
NB = BC // P             # 8 path blocks
WIDTH = 32
NG = 4                   # feature groups on partitions
NH = 3                   # hidden layers
C = BC * NSTEP // NG     # 32768 point-columns per group
CC = 512                 # chunk columns
NCHUNK = C // CC         # 64
T0, T1 = 0.0, 1.0
MU, SIG = 1.0, 1.0
DT = (T1 - T0) / NSTEP
SQDT = float(np.sqrt(DT))

F32 = mybir.dt.float32
AF = mybir.ActivationFunctionType
ALU = mybir.AluOpType

# stream dtype for hidden activations / tangents (flip to bfloat16 for speed)
SD = mybir.dt.float16

_CACHE = {}
import os
DBG_STAGE = os.environ.get("KDBG_STAGE", "full")
DBG_NCHUNK = int(os.environ.get("KDBG_NCHUNK", "0")) or None
DBG_REPS = int(os.environ.get("KDBG_REPS", "1"))


def _legalize_waits(nc):
    """Split long on_wait lists into standalone single/dual-wait NoOps.

    This walrus rejects instructions whose sync_info carries more waits
    than the ISA encoding holds (1 for DMA descriptors, ~2 for compute /
    ctrl ops). Tile emits up to one wait per logical processor (27), so
    spill the excess onto NoOps on the same engine queue, which execute
    in order before the real instruction.
    """
    ctr = 0
    for bb in nc.main_func.blocks:
        out = []
        for ins in bb.instructions:
            si = ins.sync_info
            if si is not None and si.on_wait:
                # this walrus encodes exactly one sync wait per instruction
                limit = 1
                waits = list(si.on_wait)
                if len(waits) > limit:
                    spill, keep = waits[:-limit], waits[-limit:]
                    for w in spill:
                        ctr += 1
                        nop = mybir.InstNoOp(
                            name=f"waitnop_{ctr}", ins=[], outs=[]
                        )
                        nop.engine = ins.engine
                        nop.sync_info = mybir.SyncInfo(on_wait=[w], on_update=[])
                        out.append(nop)
                    si.on_wait = keep
            out.append(ins)
        bb.instructions = out


def _build_program():
    nc = bass.Bass()

    # ---- dram parameters (per-core inputs) ----
    rn_d = nc.declare_dram_parameter("rn_sg", [P, NB * NSTEP], F32, isOutput=False)
    trow_d = nc.declare_dram_parameter("trow", [P, 2, C // P], SD, isOutput=False)
    lhsT0_d = nc.declare_dram_parameter("lhsT0", [5 * NG, P], SD, isOutput=False)
    lhsTg_d = nc.declare_dram_parameter("lhsTg", [5 * NG, P], SD, isOutput=False)
    lhsTu_d = nc.declare_dram_parameter("lhsTu", [5 * NG, P], SD, isOutput=False)
    lhsTh_d = nc.declare_dram_parameter("lhsTh", [NH, P, P], SD, isOutput=False)
    lhsTf_d = nc.declare_dram_parameter("lhsTf", [P, NG], SD, isOutput=False)
    bias_d = nc.declare_dram_parameter("bias", [P, 4, 2], F32, isOutput=False)
    bfh_d = nc.declare_dram_parameter("bfh", [P, 1], F32, isOutput=False)
    out_d = nc.declare_dram_parameter("yT", [BC, 2], F32, isOutput=True)

    with tile.TileContext(nc) as tc:
        with (
            tc.tile_pool(name="const", bufs=1) as cpool,
            tc.tile_pool(name="sg", bufs=1) as sgpool,
            tc.tile_pool(name="work", bufs=2) as wpool,
            tc.tile_pool(name="psum", bufs=8, space="PSUM") as pspool,
        ):
          for _rep in range(DBG_REPS):
            # ---- load constants ----
              lhsT0 = cpool.tile([5 * NG, P], SD, tag="lhsT0")
              lhsTg = cpool.tile([5 * NG, P], SD, tag="lhsTg")
              lhsTu = cpool.tile([5 * NG, P], SD, tag="lhsTu")
              lhsTh = [
                  cpool.tile([P, P], SD, tag=f"lhsTh{l}", name=f"lhsTh{l}")
                  for l in range(NH)
              ]
              lhsTf = cpool.tile([P, NG], SD, tag="lhsTf")
              bias = cpool.tile([P, 4, 2], F32, tag="bias")
              bfh = cpool.tile([P, 1], F32, tag="bfh")
              nc.sync.dma_start(lhsT0[:], lhsT0_d[:])
              nc.sync.dma_start(lhsTg[:], lhsTg_d[:])
              nc.sync.dma_start(lhsTu[:], lhsTu_d[:])
              for l in range(NH):
                  nc.sync.dma_start(lhsTh[l][:], lhsTh_d[l])
              nc.sync.dma_start(lhsTf[:], lhsTf_d[:])
              nc.sync.dma_start(bias[:], bias_d[:])
              nc.sync.dma_start(bfh[:], bfh_d[:])
              # bias AP views [P, 1]: bias[:, l, 0] = b tiled, [:, l, 1] = 0.5*b
              def bias_r(l, h):
                  return bias[:, l, h : h + 1]

              # ---- stage A: sgrid GBM math ----
              rs = sgpool.tile([P, NB, NSTEP], F32, tag="rs")
              nc.sync.dma_start(rs[:], rn_d[:].rearrange("p (b n) -> p b n", b=NB))
              dW = sgpool.tile([P, NB, NSTEP], F32, tag="dW")
              nc.vector.tensor_scalar_mul(dW[:], rs[:], SQDT)
              m = sgpool.tile([P, NB, NSTEP], F32, tag="m")
              # q1 = dW^2 (reuse rs as scratch)
              nc.vector.tensor_mul(rs[:], dW[:], dW[:])
              # m = (q1 * 0.5*SIG^2) + SIG*dW   (SIG == 1)
              nc.vector.scalar_tensor_tensor(
                  m[:], rs[:], 0.5 * SIG * SIG, dW[:], ALU.mult, ALU.add
              )
              c0 = 1.0 + MU * DT - 0.5 * SIG * SIG * DT
              nc.vector.tensor_scalar_add(m[:], m[:], c0)

              # prefix product per block: sfull[:, b, 0] = 1; [:, b, 1+k] = prod
              sfull = sgpool.tile([P, NB, NSTEP + 1], F32, tag="sfull")
              nc.vector.memset(sfull[:, :, 0:1], 1.0)
              for b in range(NB):
                  nc.vector.tensor_tensor_scan(
                      sfull[:, b, 1 : NSTEP + 1],
                      m[:, b, :],
                      m[:, b, :],
                      1.0,
                      ALU.mult,
                      ALU.bypass,
                  )
              sN = sfull[:, :, 0:NSTEP]

              # Ds = (m - 1) * sN ; sdW = sN * dW   (bf16 copies for the rhs repack)
              Ds = sgpool.tile([P, NB, NSTEP], SD, tag="Ds")
              nc.vector.scalar_tensor_tensor(Ds[:], m[:], 1.0, sN, ALU.subtract, ALU.mult)
              sdW = sgpool.tile([P, NB, NSTEP], SD, tag="sdW")
              nc.vector.tensor_tensor(sdW[:], sN, dW[:], ALU.mult)
              sN_b = sgpool.tile([P, NB, NSTEP], SD, tag="sN_b")
              nc.vector.tensor_copy(sN_b[:], sN)

              run_B = DBG_STAGE in ("B", "C", "full")
              run_C = DBG_STAGE in ("C", "full")
              # ---- stage B: rhs0 assembly ----
              rhs0 = sgpool.tile([5 * NG, C], SD, tag="rhs0")
              trow = sgpool.tile([P, 2, C // P], SD, tag="trow")
              if not run_B:
                  nc.vector.memset(rhs0[0:1, 0:4], 0.0)
              if run_B:
                nc.sync.dma_start(trow[:], trow_d[:])
                for g in range(NG):
                  # t rows + ones rows: DMA-from-DMA (single queue dep)
                  dst = rhs0[5 * g : 5 * g + 1, :].rearrange(
                      "one (p c) -> one p c", p=P
                  )
                  nc.sync.dma_start(dst, trow[:, 0, :])
                  dst = rhs0[5 * g + 3 : 5 * g + 4, :].rearrange(
                      "one (p c) -> one p c", p=P
                  )
                  nc.sync.dma_start(dst, trow[:, 1, :])
                scr1 = cpool.tile([1, 4], SD, tag="scr1")
                nc.sync.dma_start(scr1[:, 0:1], rhs0[5 * (NG - 1) + 3 : 5 * (NG - 1) + 4, 0:1])
                for g in range(NG):
                  dst = rhs0[5 * g + 1 : 5 * g + 2, :].rearrange(
                      "one (q b n) -> one q b n", q=32, b=NB
                  )
                  nc.sync.dma_start(dst, sN_b[32 * g : 32 * (g + 1), :, :])
                  dst = rhs0[5 * g + 2 : 5 * g + 3, :].rearrange(
                      "one (q b n) -> one q b n", q=32, b=NB
                  )
                  nc.sync.dma_start(dst, Ds[32 * g : 32 * (g + 1), :, :])
                  dst = rhs0[5 * g + 4 : 5 * g + 5, :].rearrange(
                      "one (q b n) -> one q b n", q=32, b=NB
                  )
                  nc.sync.dma_start(dst, sdW[32 * g : 32 * (g + 1), :, :])
                # end run_B

              # ---- zf staging for stage D ----
              # zf_st[4s:4s+4, ci*CC:(ci+1)*CC] accumulates stream s chunks;
              # PSUM cannot be DMA'd, so chunks are copied out by DVE/ACT.
              zf_st = sgpool.tile([P, C], SD, tag="zf_st")  # stream s lives at partitions [32s, 32s+4)
              zf_sg = [
                  sgpool.tile([P, NB, NSTEP], SD, tag=f"zf_sg{s}", name=f"zf_sg{s}")
                  for s in range(4)
              ]

              # ---- stage C: chunked MLP jet evaluation ----
              nchunk = DBG_NCHUNK or NCHUNK
              for ci in (range(nchunk) if run_C else []):
                  rv = rhs0[:, ci * CC : (ci + 1) * CC]
                  Z0 = pspool.tile([P, CC], F32, tag="ps")
                  nc.tensor.matmul(Z0[:], lhsT0[:], rv, start=True, stop=True)
                  Mg = pspool.tile([P, CC], F32, tag="ps")
                  nc.tensor.matmul(Mg[:], lhsTg[:], rv, start=True, stop=True)
                  Mu = pspool.tile([P, CC], F32, tag="ps")
                  nc.tensor.matmul(Mu[:], lhsTu[:], rv, start=True, stop=True)

                  # layer-0 elementwise
                  s1 = wpool.tile([P, CC], SD, tag="s1")
                  nc.scalar.activation(
                      s1[:], Z0[:], AF.Derivative_silu, bias=bias_r(0, 0)
                  )
                  T = wpool.tile([P, CC], SD, tag="T")
                  nc.scalar.activation(
                      T[:], Z0[:], AF.Tanh, bias=bias_r(0, 1), scale=0.5
                  )
                  A = wpool.tile([P, CC], SD, tag="A")
                  nc.scalar.activation(A[:], Mu[:], AF.Square)
                  sig = wpool.tile([P, CC], SD, tag="sig")
                  nc.vector.tensor_scalar(sig[:], T[:], 0.5, 0.5, ALU.mult, ALU.add)
                  a = wpool.tile([P, CC], SD, tag="a")
                  nc.vector.scalar_tensor_tensor(
                      a[:], Z0[:], bias_r(0, 0), sig[:], ALU.add, ALU.mult
                  )
                  G = wpool.tile([P, CC], SD, tag="G")
                  nc.vector.tensor_tensor(G[:], s1[:], T[:], ALU.mult)
                  s2p = wpool.tile([P, CC], SD, tag="s2p")
                  nc.vector.scalar_tensor_tensor(
                      s2p[:], T[:], 0.5, G[:], ALU.mult, ALU.subtract
                  )
                  u = wpool.tile([P, CC], SD, tag="u")
                  nc.vector.tensor_tensor(u[:], s1[:], Mu[:], ALU.mult)
                  gm = wpool.tile([P, CC], SD, tag="gm")
                  nc.vector.tensor_tensor(gm[:], s1[:], Mg[:], ALU.mult)
                  v = wpool.tile([P, CC], SD, tag="v")
                  nc.vector.scalar_tensor_tensor(
                      v[:], s2p[:], 0.5, A[:], ALU.add, ALU.mult
                  )

                  # hidden layers
                  for l in range(NH):
                      Zp = pspool.tile([P, CC], F32, tag="ps")
                      nc.tensor.matmul(Zp[:], lhsTh[l][:], a[:], start=True, stop=True)
                      Zu = pspool.tile([P, CC], F32, tag="ps")
                      nc.tensor.matmul(Zu[:], lhsTh[l][:], u[:], start=True, stop=True)
                      Zg = pspool.tile([P, CC], F32, tag="ps")
                      nc.tensor.matmul(Zg[:], lhsTh[l][:], gm[:], start=True, stop=True)
                      Zv = pspool.tile([P, CC], F32, tag="ps")
                      nc.tensor.matmul(Zv[:], lhsTh[l][:], v[:], start=True, stop=True)

                      s1 = wpool.tile([P, CC], SD, tag="s1")
                      nc.scalar.activation(
                          s1[:], Zp[:], AF.Derivative_silu, bias=bias_r(l + 1, 0)
                      )
                      T = wpool.tile([P, CC], SD, tag="T")
                      nc.scalar.activation(
                          T[:], Zp[:], AF.Tanh, bias=bias_r(l + 1, 1), scale=0.5
                      )
                      A = wpool.tile([P, CC], SD, tag="A")
                      nc.scalar.activation(A[:], Zu[:], AF.Square)
                      sig = wpool.tile([P, CC], SD, tag="sig")
                      nc.vector.tensor_scalar(
                          sig[:], T[:], 0.5, 0.5, ALU.mult, ALU.add
                      )
                      a = wpool.tile([P, CC], SD, tag="a")
                      nc.vector.scalar_tensor_tensor(
                          a[:], Zp[:], bias_r(l + 1, 0), sig[:], ALU.add, ALU.mult
                      )
                      G = wpool.tile([P, CC], SD, tag="G")
                      nc.vector.tensor_tensor(G[:], s1[:], T[:], ALU.mult)
                      s2p = wpool.tile([P, CC], SD, tag="s2p")
                      nc.vector.scalar_tensor_tensor(
                          s2p[:], T[:], 0.5, G[:], ALU.mult, ALU.subtract
                      )
                      u = wpool.tile([P, CC], SD, tag="u")
                      nc.vector.tensor_tensor(u[:], s1[:], Zu[:], ALU.mult)
                      gm = wpool.tile([P, CC], SD, tag="gm")
                      nc.vector.tensor_tensor(gm[:], s1[:], Zg[:], ALU.mult)
                      q = wpool.tile([P, CC], SD, tag="q")
                      nc.vector.tensor_tensor(q[:], s1[:], Zv[:], ALU.mult)
                      Bq = wpool.tile([P, CC], SD, tag="Bq")
                      nc.vector.scalar_tensor_tensor(
                          Bq[:], s2p[:], 0.5, A[:], ALU.add, ALU.mult
                      )
                      v = wpool.tile([P, CC], SD, tag="v")
                      nc.vector.tensor_tensor(v[:], Bq[:], q[:], ALU.add)

                  # final layer
                  Zf = pspool.tile([NG, CC], F32, tag="ps")
                  nc.tensor.matmul(Zf[:], lhsTf[:], a[:], start=True, stop=True)
                  Zuf = pspool.tile([NG, CC], F32, tag="ps")
                  nc.tensor.matmul(Zuf[:], lhsTf[:], u[:], start=True, stop=True)
                  Zgf = pspool.tile([NG, CC], F32, tag="ps")
                  nc.tensor.matmul(Zgf[:], lhsTf[:], gm[:], start=True, stop=True)
                  Zvf = pspool.tile([NG, CC], F32, tag="ps")
                  nc.tensor.matmul(Zvf[:], lhsTf[:], v[:], start=True, stop=True)

                  # copy [4, CC] psum chunks into the [16, C] staging buffer
                  for si, Zs in enumerate((Zf, Zuf, Zgf, Zvf)):
                      dst = zf_st[32 * si : 32 * si + 4, ci * CC : (ci + 1) * CC]
                      nc.scalar.copy(dst, Zs[:])

              run_D = run_C and (DBG_NCHUNK is None)
              vT = sgpool.tile([P, NB, 1], F32, tag="vT")
              if run_D:
                  # bridge: absorb the DMA-completion wait on this queue before
                  # the repack DMAs (DMA instructions fit only one sync wait).
                  scr2 = cpool.tile([1, 4], SD, tag="scr2")
                  nc.sync.dma_start(scr2[:, 0:1], rhs0[5 * (NG - 1) + 4 : 5 * NG, 0:1])
                  # repack staging -> sgrid layout (per stream x group to keep
                  # the DMA access patterns simple: one source partition fans
                  # out to 32 destination partitions)
                  for si in range(4):
                      for g in range(NG):
                          src = zf_st[
                              32 * si + g : 32 * si + g + 1, :
                          ].rearrange("one (q b n) -> one q b n", q=32, b=NB)
                          nc.sync.dma_start(
                              zf_sg[si][32 * g : 32 * (g + 1), :, :], src
                          )

                  # ---- stage D: phi assembly + reduction ----
                  Tf = sgpool.tile([P, NB, NSTEP], F32, tag="Tf")
                  nc.scalar.activation(Tf[:], zf_sg[0][:], AF.Tanh, bias=bfh[:], scale=0.5)
                  sp = sgpool.tile([P, NB, NSTEP], F32, tag="sp")
                  nc.vector.tensor_mul(sp[:], Tf[:], Tf[:])
                  nc.vector.tensor_scalar(sp[:], sp[:], -0.25, 0.25, ALU.mult, ALU.add)
                  S = sgpool.tile([P, NB, NSTEP], F32, tag="S")
                  nc.vector.tensor_tensor(S[:], zf_sg[2][:], zf_sg[3][:], ALU.add)
                  Q = sgpool.tile([P, NB, NSTEP], F32, tag="Q")
                  nc.vector.tensor_mul(Q[:], zf_sg[1][:], zf_sg[1][:])
                  nc.vector.tensor_mul(Q[:], Q[:], Tf[:])
                  nc.vector.tensor_tensor(S[:], S[:], Q[:], ALU.subtract)
                  nc.vector.tensor_tensor(S[:], sp[:], S[:], ALU.mult)
                  nc.vector.tensor_reduce(vT[:], S[:], mybir.AxisListType.X, ALU.add)
              else:
                  nc.vector.memset(vT[:], 0.0)

              # ---- outputs ----
              if run_D:
                  scr3 = cpool.tile([1, 4], SD, tag="scr3")
                  nc.sync.dma_start(scr3[:, 0:1], zf_sg[3][0:1, 0:1, 0:1])
              yv = out_d[:].rearrange("(b p) c -> p b c", p=P)
              nc.sync.dma_start(yv[:, :, 0:1], sfull[:, :, NSTEP : NSTEP + 1])
              nc.sync.dma_start(yv[:, :, 1:2], vT[:])

    _legalize_waits(nc)
    return nc


def _prep_host(inputs):
    rnorm = np.ascontiguousarray(np.asarray(inputs["rnorm"], dtype=np.float32))
    W0 = np.asarray(inputs["W0"], dtype=np.float32)
    b0 = np.asarray(inputs["b0"], dtype=np.float32)
    Wh = np.asarray(inputs["Wh"], dtype=np.float32)
    bh = np.asarray(inputs["bh"], dtype=np.float32)
    Wf = np.asarray(inputs["Wf"], dtype=np.float32)
    bf = np.asarray(inputs["bf"], dtype=np.float32)

    sd_np = mybir.dt.np(SD)

    # t-row pattern: col j -> t = DT * (j % NSTEP), plus a ones plane
    trow = np.ones((P, 2, C // P), np.float32)
    trow[:, 0, :] = (
        DT * np.tile(np.arange(NSTEP, dtype=np.float32), C // NSTEP)
    ).reshape(P, C // P)

    # seed lhsTs [5*NG, P]
    lhsT0 = np.zeros((5 * NG, P), np.float32)
    lhsTg = np.zeros((5 * NG, P), np.float32)
    lhsTu = np.zeros((5 * NG, P), np.float32)
    for g in range(NG):
        cols = slice(32 * g, 32 * (g + 1))
        lhsT0[5 * g + 0, cols] = W0[:, 0]          # t coefficient
        lhsT0[5 * g + 1, cols] = W0[:, 1]          # s coefficient
        lhsTg[5 * g + 2, cols] = W0[:, 1]          # Ds row
        lhsTg[5 * g + 3, cols] = W0[:, 0] * DT     # ones row -> dhdt*dt
        lhsTu[5 * g + 4, cols] = W0[:, 1] * np.sqrt(0.5) * SIG

    lhsTh = np.zeros((NH, P, P), np.float32)
    for l in range(NH):
        for g in range(NG):
            blk = slice(32 * g, 32 * (g + 1))
            lhsTh[l, blk, blk] = Wh[l].T
    lhsTf = np.zeros((P, NG), np.float32)
    for g in range(NG):
        lhsTf[32 * g : 32 * (g + 1), g] = Wf[0]

    bias = np.zeros((P, 4, 2), np.float32)
    bias[:, 0, 0] = np.tile(b0, NG)
    bias[:, 0, 1] = 0.5 * bias[:, 0, 0]
    for l in range(NH):
        bias[:, l + 1, 0] = np.tile(bh[l], NG)
        bias[:, l + 1, 1] = 0.5 * bias[:, l + 1, 0]
    bfh = np.full((P, 1), 0.5 * bf[0], np.float32)

    shared = {
        "trow": trow.astype(sd_np),
        "lhsT0": lhsT0.astype(sd_np),
        "lhsTg": lhsTg.astype(sd_np),
        "lhsTu": lhsTu.astype(sd_np),
        "lhsTh": lhsTh.astype(sd_np),
        "lhsTf": lhsTf.astype(sd_np),
        "bias": bias,
        "bfh": bfh,
    }

    in_maps = []
    for core in range(NCORE):
        shard = rnorm[core * BC : (core + 1) * BC]          # [1024, 128]
        sg = np.ascontiguousarray(
            shard.reshape(NB, P, NSTEP).transpose(1, 0, 2).reshape(P, NB * NSTEP)
        )
        in_maps.append({"rn_sg": sg, **shared})
    return in_maps


last_perf = {}


def kernel(trace=False, **inputs) -> np.ndarray:
    if "nc" not in _CACHE:
        _CACHE["nc"] = _build_program()
    nc = _CACHE["nc"]
    in_maps = _prep_host(inputs)
    res = run_bass_kernel_spmd(nc, in_maps, list(range(NCORE)), trace=trace)
    last_perf["exec_time_ns"] = res.exec_time_ns
    out = np.empty((B, 2), np.float32)
    for core in range(NCORE):
        yt = res.results[core]["yT"]                        # [1024, 2]
        out[core * BC : (core + 1) * BC] = yt
    return out



# revision 8
# speedup vs baseline: 1.7200x; 1.7200x over previous
"""Trainium2 Bass kernel for the deep-hedging Milstein SDE loss.

Math: the reference scan has closed-form structure. With y = [s, v]:
  s_{n+1} = s_n * m_n,  m_n = 1 + MU*dt + SIG*dW_n + 0.5*SIG^2*(dW_n^2 - dt)
  v_{n+1} = v_n + dhdt*dt + dhds*(s_{n+1}-s_n) + 0.5*SIG^2*s_n^2*dW_n^2*dhdss
where (dhdt, dhds, dhdss) are derivatives of the holding MLP h(t, s) at
(t_n, s_n).  The scan collapses to:
  1. prefix-product along steps for s_n (tensor_tensor_scan)
  2. one fully-batched forward-mode jet evaluation of the MLP over all
     B*N points with 3 tangent streams
  3. a per-path reduction over steps.

Layout per core (1024 paths, path_local = b*128 + pi for partition pi,
block b):  MLP groups g = pi % 4 (q = pi // 4), so chunk q's rhs comes
from 4 CONTIGUOUS partitions S3[4q:4q+4] via one cheap DMA per chunk
(dest-partition-bytes is what the DMA cost model charges; single-row
gathers are ~32x more expensive).  rhs rows: p = 3g+s for the per-group
streams (sN, Ds, sdW), rows 12/13 = shared static t-row / ones-row.
Final-layer zf staging rows p = 4s+g; stage D unpacks per (s, g) to
sgrid layout pi'' = 32g + q, host un-permutes the vT column.

Engine balance per chunk (CoreSim cost model, CC=1024). GPSIMD (Pool)
cannot read PSUM, so ACT evacuates Zu once per layer (Zu16) and Pool
computes A=Zu16^2 / u=s1*Zu16 from SBUF; v=Bq+q is folded into the next
layer's matmul as two accumulating matmuls (PE has slack):
  ACT: s1, T, Zu16 per layer, Tf, U2, GV                   (~15.6us)
  DVE: sig/cmh (4x ts), a (stt psum), g, q (tt psum)       (~15.7us)
  Pool: A, u, s2p = T*(0.5-s1), Bq, v_L0                   (~14.0us)
"""

import numpy as np

import concourse.bass as bass
import concourse.mybir as mybir
from concourse import tile
from concourse.bass_utils import run_bass_kernel_spmd


# problem constants (hardcoded per spec)
B = 8192
NSTEP = 128
NCORE = 8
BC = B // NCORE          # 1024 paths per core
P = 128                  # partitions
NB = BC // P             # 8 path blocks
WIDTH = 32
NG = 4                   # feature groups on partitions
NH = 3                   # hidden layers
NQ = 32                  # within-group path index == chunk index
CC = NB * NSTEP          # 1024 point-columns per chunk
NCHUNK = NQ              # 32
NRB = 3                  # rhs double-buffers
T0, T1 = 0.0, 1.0
MU, SIG = 1.0, 1.0
DT = (T1 - T0) / NSTEP
SQDT = float(np.sqrt(DT))

F32 = mybir.dt.float32
AF = mybir.ActivationFunctionType
ALU = mybir.AluOpType

SD = mybir.dt.float16

_CACHE = {}


def _legalize_waits(nc):
    """Split long on_wait lists into standalone single-wait NoOps.

    This walrus rejects instructions whose sync_info carries more waits
    than the ISA encoding holds; spill the excess onto NoOps on the same
    engine queue, which execute in order before the real instruction.
    """
    ctr = 0
    for bb in nc.main_func.blocks:
        out = []
        for ins in bb.instructions:
            si = ins.sync_info
            if si is not None and si.on_wait:
                limit = 1
                waits = list(si.on_wait)
                if len(waits) > limit:
                    spill, keep = waits[:-limit], waits[-limit:]
                    for w in spill:
                        ctr += 1
                        nop = mybir.InstNoOp(
                            name=f"waitnop_{ctr}", ins=[], outs=[]
                        )
                        nop.engine = ins.engine
                        nop.sync_info = mybir.SyncInfo(on_wait=[w], on_update=[])
                        out.append(nop)
                    si.on_wait = keep
            out.append(ins)
        bb.instructions = out


def _build_program():
    nc = bass.Bass()

    rn_d = nc.declare_dram_parameter("rn_sg", [P, NB * NSTEP], F32, isOutput=False)
    trow_d = nc.declare_dram_parameter("trow", [2, CC], SD, isOutput=False)
    lhsT0_d = nc.declare_dram_parameter("lhsT0", [14, P], SD, isOutput=False)
    lhsTg_d = nc.declare_dram_parameter("lhsTg", [14, P], SD, isOutput=False)
    lhsTu_d = nc.declare_dram_parameter("lhsTu", [14, P], SD, isOutput=False)
    lhsTh_d = nc.declare_dram_parameter("lhsTh", [NH, P, P], SD, isOutput=False)
    lhsTf_d = nc.declare_dram_parameter("lhsTf", [P, NG], SD, isOutput=False)
    bias_d = nc.declare_dram_parameter("bias", [P, 4, 2], F32, isOutput=False)
    bfh_d = nc.declare_dram_parameter("bfh", [P, 1], F32, isOutput=False)
    yS_d = nc.declare_dram_parameter("yS", [P, NB], F32, isOutput=True)
    yV_d = nc.declare_dram_parameter("yV", [P, NB], F32, isOutput=True)

    with tile.TileContext(nc) as tc:
        with (
            tc.tile_pool(name="const", bufs=1) as cpool,
            tc.tile_pool(name="sg", bufs=1) as sgpool,
            tc.tile_pool(name="work", bufs=3) as wpool,
            tc.tile_pool(name="psum", bufs=4, space="PSUM") as pspool,
        ):
            # ---- load constants ----
            lhsT0 = cpool.tile([14, P], SD, tag="lhsT0")
            lhsTg = cpool.tile([14, P], SD, tag="lhsTg")
            lhsTu = cpool.tile([14, P], SD, tag="lhsTu")
            lhsTh = [
                cpool.tile([P, P], SD, tag=f"lhsTh{l}", name=f"lhsTh{l}")
                for l in range(NH)
            ]
            lhsTf = cpool.tile([P, NG], SD, tag="lhsTf")
            bias = cpool.tile([P, 4, 2], F32, tag="bias")
            bfh = cpool.tile([P, 1], F32, tag="bfh")
            nc.sync.dma_start(lhsT0[:], lhsT0_d[:])
            nc.sync.dma_start(lhsTg[:], lhsTg_d[:])
            nc.sync.dma_start(lhsTu[:], lhsTu_d[:])
            for l in range(NH):
                nc.sync.dma_start(lhsTh[l][:], lhsTh_d[l])
            nc.sync.dma_start(lhsTf[:], lhsTf_d[:])
            nc.sync.dma_start(bias[:], bias_d[:])
            nc.sync.dma_start(bfh[:], bfh_d[:])

            def bias_r(l, h):
                return bias[:, l, h : h + 1]

            # rhs chunk buffers; static rows 12 (t) / 13 (ones) filled once
            rhs_bufs = [
                cpool.tile([14, CC], SD, tag=f"rhs{k}", name=f"rhs{k}")
                for k in range(NRB)
            ]
            for k in range(NRB):
                nc.sync.dma_start(rhs_bufs[k][12:14, :], trow_d[:])

            # ---- stage A: sgrid GBM math ----
            rs = sgpool.tile([P, NB, NSTEP], F32, tag="rs")
            nc.sync.dma_start(rs[:], rn_d[:].rearrange("p (b n) -> p b n", b=NB))
            dW = sgpool.tile([P, NB, NSTEP], F32, tag="dW")
            nc.vector.tensor_scalar_mul(dW[:], rs[:], SQDT)
            m = sgpool.tile([P, NB, NSTEP], F32, tag="m")
            # q1 = dW^2 (reuse rs as scratch)
            nc.vector.tensor_mul(rs[:], dW[:], dW[:])
            nc.vector.scalar_tensor_tensor(
                m[:], rs[:], 0.5 * SIG * SIG, dW[:], ALU.mult, ALU.add
            )
            c0 = 1.0 + MU * DT - 0.5 * SIG * SIG * DT
            nc.vector.tensor_scalar_add(m[:], m[:], c0)

            sfull = sgpool.tile([P, NB, NSTEP + 1], F32, tag="sfull")
            nc.vector.memset(sfull[:, :, 0:1], 1.0)
            for b in range(NB):
                nc.vector.tensor_tensor_scan(
                    sfull[:, b, 1 : NSTEP + 1],
                    m[:, b, :],
                    m[:, b, :],
                    1.0,
                    ALU.mult,
                    ALU.bypass,
                )
            sN = sfull[:, :, 0:NSTEP]

            # S3[:, 0]=sN, [:, 1]=Ds=(m-1)*sN, [:, 2]=sdW=sN*dW   (f16)
            S3 = sgpool.tile([P, 3, NB, NSTEP], SD, tag="S3")
            nc.vector.tensor_copy(S3[:, 0], sN)
            nc.vector.scalar_tensor_tensor(
                S3[:, 1], m[:], 1.0, sN, ALU.subtract, ALU.mult
            )
            nc.vector.tensor_tensor(S3[:, 2], sN, dW[:], ALU.mult)

            # zf staging: compute writes must start at 32-aligned
            # partitions, so streams live at partition offsets 0 (Tf),
            # 32 (zu^2), 64 (zg+zv), rows +g within each block.
            zf_st = sgpool.tile([P, NCHUNK * CC], SD, tag="zf_st")

            HC = CC // 2

            def mm(out, lhsT, rhs):
                # PSUM banks are 2KB; a single matmul output must stay in
                # one bank, so emit one matmul per 512-col half.
                for h in range(2):
                    cs = slice(h * HC, (h + 1) * HC)
                    nc.tensor.matmul(
                        out[:, cs], lhsT[:], rhs[:, cs], start=True, stop=True
                    )

            def mm_acc(out, lhsT, rhss):
                for h in range(2):
                    cs = slice(h * HC, (h + 1) * HC)
                    for i, r in enumerate(rhss):
                        nc.tensor.matmul(
                            out[:, cs], lhsT[:], r[:, cs],
                            start=(i == 0), stop=(i == len(rhss) - 1),
                        )

            # ---- chunk loop ----
            for q in range(NCHUNK):
                rb = rhs_bufs[q % NRB]
                nc.sync.dma_start(rb[0:12, :], S3[4 * q : 4 * q + 4, :, :, :])

                # layer 0
                Z0 = pspool.tile([P, CC], F32, tag="ps", name=f"Z0_{q}")
                mm(Z0, lhsT0, rb)
                Mg = pspool.tile([P, CC], F32, tag="ps", name=f"Mg_{q}")
                mm(Mg, lhsTg, rb)
                Mu = pspool.tile([P, CC], F32, tag="ps", name=f"Mu_{q}")
                mm(Mu, lhsTu, rb)

                s1 = wpool.tile([P, CC], SD, tag="s1", name=f"s1_{q}")
                nc.scalar.activation(
                    s1[:], Z0[:], AF.Derivative_silu, bias=bias_r(0, 0)
                )
                T = wpool.tile([P, CC], SD, tag="T", name=f"T_{q}")
                nc.scalar.activation(T[:], Z0[:], AF.Tanh, bias=bias_r(0, 1), scale=0.5)
                Zu16 = wpool.tile([P, CC], SD, tag="Zu16", name=f"Mu16_{q}")
                nc.scalar.copy(Zu16[:], Mu[:])
                sig = wpool.tile([P, CC], SD, tag="sig", name=f"sig_{q}")
                nc.vector.tensor_scalar(sig[:], T[:], 0.5, 0.5, ALU.mult, ALU.add)
                A = wpool.tile([P, CC], SD, tag="A", name=f"A_{q}")
                nc.gpsimd.tensor_tensor(A[:], Zu16[:], Zu16[:], ALU.mult)
                a = wpool.tile([P, CC], SD, tag="a", name=f"a_{q}")
                nc.vector.scalar_tensor_tensor(
                    a[:], Z0[:], bias_r(0, 0), sig[:], ALU.add, ALU.mult
                )
                # silu'' = sig - s1*T
                G = wpool.tile([P, CC], SD, tag="G", name=f"G_{q}")
                nc.gpsimd.tensor_tensor(G[:], s1[:], T[:], ALU.mult)
                sil2 = wpool.tile([P, CC], SD, tag="sil2", name=f"sil2_{q}")
                nc.gpsimd.tensor_tensor(sil2[:], sig[:], G[:], ALU.subtract)
                u = wpool.tile([P, CC], SD, tag="u", name=f"u_{q}")
                nc.gpsimd.tensor_tensor(u[:], s1[:], Zu16[:], ALU.mult)
                gm = wpool.tile([P, CC], SD, tag="gm", name=f"gm_{q}")
                nc.vector.tensor_tensor(gm[:], s1[:], Mg[:], ALU.mult)
                v = wpool.tile([P, CC], SD, tag="v", name=f"v_{q}")
                nc.gpsimd.tensor_tensor(v[:], sil2[:], A[:], ALU.mult)

                # hidden layers; v = Bq + q is folded into the next
                # layer's Zv matmul as two accumulating matmuls
                Bq = qn = None
                for l in range(NH):
                    Zp = pspool.tile([P, CC], F32, tag="ps", name=f"Zp_{q}_{l}")
                    mm(Zp, lhsTh[l], a)
                    Zu = pspool.tile([P, CC], F32, tag="ps", name=f"Zu_{q}_{l}")
                    mm(Zu, lhsTh[l], u)
                    Zg = pspool.tile([P, CC], F32, tag="ps", name=f"Zg_{q}_{l}")
                    mm(Zg, lhsTh[l], gm)
                    Zv = pspool.tile([P, CC], F32, tag="ps", name=f"Zv_{q}_{l}")
                    if l == 0:
                        mm(Zv, lhsTh[l], v)
                    else:
                        mm_acc(Zv, lhsTh[l], [Bq, qn])

                    s1 = wpool.tile([P, CC], SD, tag="s1", name=f"s1_{q}_{l}")
                    nc.scalar.activation(
                        s1[:], Zp[:], AF.Derivative_silu, bias=bias_r(l + 1, 0)
                    )
                    T = wpool.tile([P, CC], SD, tag="T", name=f"T_{q}_{l}")
                    nc.scalar.activation(
                        T[:], Zp[:], AF.Tanh, bias=bias_r(l + 1, 1), scale=0.5
                    )
                    Zu16 = wpool.tile([P, CC], SD, tag="Zu16", name=f"Zu16_{q}_{l}")
                    nc.scalar.copy(Zu16[:], Zu[:])
                    sig = wpool.tile([P, CC], SD, tag="sig", name=f"sig_{q}_{l}")
                    nc.vector.tensor_scalar(sig[:], T[:], 0.5, 0.5, ALU.mult, ALU.add)
                    A = wpool.tile([P, CC], SD, tag="A", name=f"A_{q}_{l}")
                    nc.gpsimd.tensor_tensor(A[:], Zu16[:], Zu16[:], ALU.mult)
                    a = wpool.tile([P, CC], SD, tag="a", name=f"a_{q}_{l}")
                    nc.vector.scalar_tensor_tensor(
                        a[:], Zp[:], bias_r(l + 1, 0), sig[:], ALU.add, ALU.mult
                    )
                    G = wpool.tile([P, CC], SD, tag="G", name=f"G_{q}_{l}")
                    nc.gpsimd.tensor_tensor(G[:], s1[:], T[:], ALU.mult)
                    sil2 = wpool.tile([P, CC], SD, tag="sil2", name=f"sil2_{q}_{l}")
                    nc.gpsimd.tensor_tensor(sil2[:], sig[:], G[:], ALU.subtract)
                    un = wpool.tile([P, CC], SD, tag="u", name=f"u_{q}_{l}")
                    if l == 0:
                        nc.gpsimd.tensor_tensor(un[:], s1[:], Zu16[:], ALU.mult)
                    else:
                        nc.vector.tensor_tensor(un[:], s1[:], Zu16[:], ALU.mult)
                    gn = wpool.tile([P, CC], SD, tag="gm", name=f"g_{q}_{l}")
                    nc.vector.tensor_tensor(gn[:], s1[:], Zg[:], ALU.mult)
                    qn = wpool.tile([P, CC], SD, tag="q", name=f"q_{q}_{l}")
                    nc.vector.tensor_tensor(qn[:], s1[:], Zv[:], ALU.mult)
                    Bq = wpool.tile([P, CC], SD, tag="Bq", name=f"Bq_{q}_{l}")
                    nc.gpsimd.tensor_tensor(Bq[:], sil2[:], A[:], ALU.mult)
                    u, gm = un, gn

                # final layer: Zf (a), Zuf (u), Zgv (gm + Bq + q accumulated)
                Zf = pspool.tile([NG, CC], F32, tag="ps", name=f"Zf_{q}")
                mm(Zf, lhsTf, a)
                Zuf = pspool.tile([NG, CC], F32, tag="ps", name=f"Zuf_{q}")
                mm(Zuf, lhsTf, u)
                Zgv = pspool.tile([NG, CC], F32, tag="ps", name=f"Zgv_{q}")
                mm_acc(Zgv, lhsTf, [gm, Bq, qn])

                cols = slice(q * CC, (q + 1) * CC)
                # Tf = tanh(0.5 zf + 0.5 bf) computed during evacuation
                nc.scalar.activation(
                    zf_st[0:4, cols], Zf[:], AF.Tanh, bias=bfh[0:4, :], scale=0.5
                )
                nc.scalar.activation(zf_st[32:36, cols], Zuf[:], AF.Square)
                nc.scalar.copy(zf_st[64:68, cols], Zgv[:])

            # ---- stage D: unpack + phi assembly + reduction ----
            TfS = sgpool.tile([P, NB, NSTEP], SD, tag="TfS")
            U2S = sgpool.tile([P, NB, NSTEP], SD, tag="U2S")
            GVS = sgpool.tile([P, NB, NSTEP], SD, tag="GVS")
            for si, dstt in enumerate((TfS, U2S, GVS)):
                for g in range(NG):
                    src = zf_st[32 * si + g : 32 * si + g + 1, :].rearrange(
                        "one (q b n) -> one q b n", q=NQ, b=NB
                    )
                    nc.sync.dma_start(dstt[32 * g : 32 * (g + 1), :, :], src)

            Q = wpool.tile([P, NB, NSTEP], SD, tag="dQ")
            nc.vector.tensor_tensor(Q[:], U2S[:], TfS[:], ALU.mult)
            S = wpool.tile([P, NB, NSTEP], SD, tag="dS")
            nc.vector.tensor_tensor(S[:], GVS[:], Q[:], ALU.subtract)
            E = wpool.tile([P, NB, NSTEP], SD, tag="dE")
            nc.gpsimd.tensor_tensor(E[:], TfS[:], TfS[:], ALU.mult)
            sp = wpool.tile([P, NB, NSTEP], SD, tag="dsp")
            nc.vector.tensor_scalar(sp[:], E[:], -0.25, 0.25, ALU.mult, ALU.add)
            S2 = wpool.tile([P, NB, NSTEP], SD, tag="dS2")
            nc.vector.tensor_tensor(S2[:], S[:], sp[:], ALU.mult)
            vT = sgpool.tile([P, NB, 1], F32, tag="vT")
            nc.vector.tensor_reduce(vT[:], S2[:], mybir.AxisListType.X, ALU.add)

            # ---- outputs ----
            nc.sync.dma_start(yS_d[:], sfull[:, :, NSTEP : NSTEP + 1])
            nc.sync.dma_start(yV_d[:], vT[:])

    _legalize_waits(nc)
    return nc


def _prep_host(inputs):
    rnorm = np.ascontiguousarray(np.asarray(inputs["rnorm"], dtype=np.float32))
    W0 = np.asarray(inputs["W0"], dtype=np.float32)
    b0 = np.asarray(inputs["b0"], dtype=np.float32)
    Wh = np.asarray(inputs["Wh"], dtype=np.float32)
    bh = np.asarray(inputs["bh"], dtype=np.float32)
    Wf = np.asarray(inputs["Wf"], dtype=np.float32)
    bf = np.asarray(inputs["bf"], dtype=np.float32)

    sd_np = mybir.dt.np(SD)

    # static rhs rows: t-row (period 128), ones-row
    trow = np.ones((2, CC), np.float32)
    trow[0, :] = DT * np.tile(np.arange(NSTEP, dtype=np.float32), CC // NSTEP)

    # lhsT seeds [14, P]: row 3g+s for per-group streams, 12 t, 13 ones
    lhsT0 = np.zeros((14, P), np.float32)
    lhsTg = np.zeros((14, P), np.float32)
    lhsTu = np.zeros((14, P), np.float32)
    for g in range(NG):
        cols = slice(32 * g, 32 * (g + 1))
        lhsT0[3 * g + 0, cols] = W0[:, 1]              # s coefficient
        lhsTg[3 * g + 1, cols] = W0[:, 1]              # Ds row
        lhsTu[3 * g + 2, cols] = W0[:, 1] * np.sqrt(0.5) * SIG
        lhsT0[12, cols] = W0[:, 0]                     # t row
        lhsTg[13, cols] = W0[:, 0] * DT                # ones row -> dhdt*dt

    lhsTh = np.zeros((NH, P, P), np.float32)
    for l in range(NH):
        for g in range(NG):
            blk = slice(32 * g, 32 * (g + 1))
            lhsTh[l, blk, blk] = Wh[l].T
    lhsTf = np.zeros((P, NG), np.float32)
    for g in range(NG):
        lhsTf[32 * g : 32 * (g + 1), g] = Wf[0]

    bias = np.zeros((P, 4, 2), np.float32)
    bias[:, 0, 0] = np.tile(b0, NG)
    bias[:, 0, 1] = 0.5 * bias[:, 0, 0]
    for l in range(NH):
        bias[:, l + 1, 0] = np.tile(bh[l], NG)
        bias[:, l + 1, 1] = 0.5 * bias[:, l + 1, 0]
    bfh = np.full((P, 1), 0.5 * bf[0], np.float32)

    shared = {
        "trow": trow.astype(sd_np),
        "lhsT0": lhsT0.astype(sd_np),
        "lhsTg": lhsTg.astype(sd_np),
        "lhsTu": lhsTu.astype(sd_np),
        "lhsTh": lhsTh.astype(sd_np),
        "lhsTf": lhsTf.astype(sd_np),
        "bias": bias,
        "bfh": bfh,
    }

    in_maps = []
    for core in range(NCORE):
        shard = rnorm[core * BC : (core + 1) * BC]          # [1024, 128]
        sg = np.ascontiguousarray(
            shard.reshape(NB, P, NSTEP).transpose(1, 0, 2).reshape(P, NB * NSTEP)
        )
        in_maps.append({"rn_sg": sg, **shared})
    return in_maps


# vT row pi'' = 32g+q holds path b*128 + (4q+g)
_VPERM = np.empty(P, np.int64)
for _pi in range(P):
    _g, _q = divmod(_pi, 32)
    _VPERM[_pi] = 4 * _q + _g

last_perf = {}


def kernel(trace=False, **inputs) -> np.ndarray:
    if "nc" not in _CACHE:
        _CACHE["nc"] = _build_program()
    nc = _CACHE["nc"]
    in_maps = _prep_host(inputs)
    res = run_bass_kernel_spmd(nc, in_maps, list(range(NCORE)), trace=trace)
    last_perf["exec_time_ns"] = res.exec_time_ns
    out = np.empty((B, 2), np.float32)
    for core in range(NCORE):
        yS = res.results[core]["yS"]                        # [128, 8]
        yV = res.results[core]["yV"]                        # [128, 8]
        blk = out[core * BC : (core + 1) * BC]
        blk[:, 0] = yS.T.reshape(-1)
        v = np.empty((NB, P), np.float32)
        v[:, _VPERM] = yV.T
        blk[:, 1] = v.reshape(-1)
    return out


# revision 10
# speedup vs baseline: 2.8848x; 1.6772x over previous
"""Trainium2 Bass kernel for the deep-hedging Milstein SDE loss.

Math: the reference scan has closed-form structure. With y = [s, v]:
  s_{n+1} = s_n * m_n,  m_n = 1 + MU*dt + SIG*dW_n + 0.5*SIG^2*(dW_n^2 - dt)
  v_{n+1} = v_n + dhdt*dt + dhds*(s_{n+1}-s_n) + 0.5*SIG^2*s_n^2*dW_n^2*dhdss
where (dhdt, dhds, dhdss) are derivatives of the holding MLP h(t, s) at
(t_n, s_n).  The scan collapses to:
  1. prefix-product along steps for s_n (tensor_tensor_scan)
  2. one fully-batched forward-mode jet evaluation of the MLP over all
     B*N points with 3 tangent streams
  3. a per-path reduction over steps.

Layout per core (1024 paths, path_local = b*128 + pi for partition pi,
block b):  MLP groups g = pi % 4 (q = pi // 4), so chunk q's rhs comes
from 4 CONTIGUOUS partitions S3[4q:4q+4] via one cheap DMA per chunk
(the DMA cost model charges max bytes-per-destination-partition;
single-row gathers are ~32x more expensive).  rhs rows: p = 3g+s for
the per-group streams (sN, Ds, sdW), rows 12/13 = shared static
t-row / ones-row.  Final-layer outputs bounce through a per-chunk zc
tile (compute writes need 32-aligned partition starts) and DMA to
sgrid tiles at partitions [4q:4q+4] - same path order as stage A.

Engine notes (CoreSim cost model + walrus ISA constraints):
  - GPSIMD (Pool) cannot read PSUM and only runs TensorTensor; it gets
    all-SBUF f16 multiplies (A, G, sil2, Bq, u, v) at a flat 878ns.
  - ACT evacuates each layer's Zu once (Zu16, scalar.copy) feeding
    Pool's A and u; silu'' = sig - s1*T avoids TensorScalarPtr on Pool.
  - v = Bq + q is folded into the next layer's Zv matmul as two
    accumulating matmuls (PE has slack).
  - Chunks are software-pipelined with a 5-stage skew (L0, h0, h1, h2,
    final) so in-order engine queues interleave 5 independent chunks.
"""

import numpy as np

import concourse.bass as bass
import concourse.mybir as mybir
from concourse import tile
from concourse.bass_utils import run_bass_kernel_spmd


# problem constants (hardcoded per spec)
B = 8192
NSTEP = 128
NCORE = 8
BC = B // NCORE          # 1024 paths per core
P = 128                  # partitions
NB = BC // P             # 8 path blocks
WIDTH = 32
NG = 4                   # feature groups on partitions
NH = 3                   # hidden layers
NQ = 32                  # within-group path index == chunk index
CC = NB * NSTEP          # 1024 point-columns per chunk
NCHUNK = NQ              # 32
NRB = 3                  # rhs buffers
T0, T1 = 0.0, 1.0
MU, SIG = 1.0, 1.0
DT = (T1 - T0) / NSTEP
SQDT = float(np.sqrt(DT))

F32 = mybir.dt.float32
AF = mybir.ActivationFunctionType
ALU = mybir.AluOpType

SD = mybir.dt.float16

_CACHE = {}


def _legalize_waits(nc):
    """Split long on_wait lists into standalone single-wait NoOps.

    This walrus rejects instructions whose sync_info carries more waits
    than the ISA encoding holds; spill the excess onto NoOps on the same
    engine queue, which execute in order before the real instruction.
    """
    ctr = 0
    for bb in nc.main_func.blocks:
        out = []
        for ins in bb.instructions:
            si = ins.sync_info
            if si is not None and si.on_wait:
                limit = 1
                waits = list(si.on_wait)
                if len(waits) > limit:
                    spill, keep = waits[:-limit], waits[-limit:]
                    for w in spill:
                        ctr += 1
                        nop = mybir.InstNoOp(
                            name=f"waitnop_{ctr}", ins=[], outs=[]
                        )
                        nop.engine = ins.engine
                        nop.sync_info = mybir.SyncInfo(on_wait=[w], on_update=[])
                        out.append(nop)
                    si.on_wait = keep
            out.append(ins)
        bb.instructions = out


def _build_program():
    nc = bass.Bass()

    rn_d = nc.declare_dram_parameter("rn_sg", [P, NB * NSTEP], F32, isOutput=False)
    trow_d = nc.declare_dram_parameter("trow", [2, CC], SD, isOutput=False)
    lhsT0_d = nc.declare_dram_parameter("lhsT0", [14, P], SD, isOutput=False)
    lhsTg_d = nc.declare_dram_parameter("lhsTg", [14, P], SD, isOutput=False)
    lhsTu_d = nc.declare_dram_parameter("lhsTu", [14, P], SD, isOutput=False)
    lhsTh_d = nc.declare_dram_parameter("lhsTh", [NH, P, P], SD, isOutput=False)
    lhsTf_d = nc.declare_dram_parameter("lhsTf", [P, NG], SD, isOutput=False)
    bias_d = nc.declare_dram_parameter("bias", [P, 4, 2], F32, isOutput=False)
    bfh_d = nc.declare_dram_parameter("bfh", [P, 1], F32, isOutput=False)
    yS_d = nc.declare_dram_parameter("yS", [P, NB], F32, isOutput=True)
    yV_d = nc.declare_dram_parameter("yV", [P, NB], F32, isOutput=True)

    HC = CC // 2

    with tile.TileContext(nc) as tc:
        with (
            tc.tile_pool(name="const", bufs=1) as cpool,
            tc.tile_pool(name="sg", bufs=1) as sgpool,
            tc.tile_pool(name="work", bufs=5) as wpool,
            tc.tile_pool(name="zcp", bufs=3) as zcpool,
            tc.tile_pool(name="psum", bufs=4, space="PSUM") as pspool,
        ):
            # ---- load constants ----
            lhsT0 = cpool.tile([14, P], SD, tag="lhsT0")
            lhsTg = cpool.tile([14, P], SD, tag="lhsTg")
            lhsTu = cpool.tile([14, P], SD, tag="lhsTu")
            lhsTh = [
                cpool.tile([P, P], SD, tag=f"lhsTh{l}", name=f"lhsTh{l}")
                for l in range(NH)
            ]
            lhsTf = cpool.tile([P, NG], SD, tag="lhsTf")
            bias = cpool.tile([P, 4, 2], F32, tag="bias")
            bfh = cpool.tile([P, 1], F32, tag="bfh")
            nc.sync.dma_start(lhsT0[:], lhsT0_d[:])
            nc.sync.dma_start(lhsTg[:], lhsTg_d[:])
            nc.sync.dma_start(lhsTu[:], lhsTu_d[:])
            for l in range(NH):
                nc.sync.dma_start(lhsTh[l][:], lhsTh_d[l])
            nc.sync.dma_start(lhsTf[:], lhsTf_d[:])
            nc.sync.dma_start(bias[:], bias_d[:])
            nc.sync.dma_start(bfh[:], bfh_d[:])

            def bias_r(l, h):
                return bias[:, l, h : h + 1]

            # rhs chunk buffers; static rows 12 (t) / 13 (ones) filled once
            rhs_bufs = [
                cpool.tile([14, CC], SD, tag=f"rhs{k}", name=f"rhs{k}")
                for k in range(NRB)
            ]
            for k in range(NRB):
                nc.sync.dma_start(rhs_bufs[k][12:14, :], trow_d[:])

            # ---- stage A: sgrid GBM math ----
            rs = sgpool.tile([P, NB, NSTEP], F32, tag="rs")
            nc.sync.dma_start(rs[:], rn_d[:].rearrange("p (b n) -> p b n", b=NB))
            dW = sgpool.tile([P, NB, NSTEP], F32, tag="dW")
            nc.vector.tensor_scalar_mul(dW[:], rs[:], SQDT)
            m = sgpool.tile([P, NB, NSTEP], F32, tag="m")
            nc.vector.tensor_mul(rs[:], dW[:], dW[:])
            nc.vector.scalar_tensor_tensor(
                m[:], rs[:], 0.5 * SIG * SIG, dW[:], ALU.mult, ALU.add
            )
            c0 = 1.0 + MU * DT - 0.5 * SIG * SIG * DT
            nc.vector.tensor_scalar_add(m[:], m[:], c0)

            sfull = sgpool.tile([P, NB, NSTEP + 1], F32, tag="sfull")
            nc.vector.memset(sfull[:, :, 0:1], 1.0)
            for b in range(NB):
                nc.vector.tensor_tensor_scan(
                    sfull[:, b, 1 : NSTEP + 1],
                    m[:, b, :],
                    m[:, b, :],
                    1.0,
                    ALU.mult,
                    ALU.bypass,
                )
            sN = sfull[:, :, 0:NSTEP]

            # S3[:, 0]=sN, [:, 1]=Ds=(m-1)*sN, [:, 2]=sdW=sN*dW   (f16)
            S3 = sgpool.tile([P, 3, NB, NSTEP], SD, tag="S3")
            nc.vector.tensor_copy(S3[:, 0], sN)
            nc.vector.scalar_tensor_tensor(
                S3[:, 1], m[:], 1.0, sN, ALU.subtract, ALU.mult
            )
            nc.vector.tensor_tensor(S3[:, 2], sN, dW[:], ALU.mult)

            # final-output sgrid tiles, filled per chunk at [4q:4q+4]
            TfS = sgpool.tile([P, NB, NSTEP], SD, tag="TfS")
            U2S = sgpool.tile([P, NB, NSTEP], SD, tag="U2S")
            GVS = sgpool.tile([P, NB, NSTEP], SD, tag="GVS")

            def mm(out, lhsT, rhs):
                # PSUM banks are 2KB; a single matmul output must stay in
                # one bank, so emit one matmul per 512-col half.
                for h in range(2):
                    cs = slice(h * HC, (h + 1) * HC)
                    nc.tensor.matmul(
                        out[:, cs], lhsT[:], rhs[:, cs], start=True, stop=True
                    )

            def mm_acc(out, lhsT, rhss):
                for h in range(2):
                    cs = slice(h * HC, (h + 1) * HC)
                    for i, r in enumerate(rhss):
                        nc.tensor.matmul(
                            out[:, cs], lhsT[:], r[:, cs],
                            start=(i == 0), stop=(i == len(rhss) - 1),
                        )

            # ---- software-pipelined chunk loop (5-stage skew) ----
            st = {}  # q -> carried stream tiles

            def elemwise(q, l, Zp, Zu, bl):
                """Shared elementwise jet update for one layer."""
                s1 = wpool.tile([P, CC], SD, tag="s1", name=f"s1_{q}_{l}")
                nc.scalar.activation(
                    s1[:], Zp[:], AF.Derivative_silu, bias=bias_r(bl, 0)
                )
                T = wpool.tile([P, CC], SD, tag="T", name=f"T_{q}_{l}")
                nc.scalar.activation(
                    T[:], Zp[:], AF.Tanh, bias=bias_r(bl, 1), scale=0.5
                )
                Zu16 = wpool.tile([P, CC], SD, tag="Zu16", name=f"Zu16_{q}_{l}")
                nc.scalar.copy(Zu16[:], Zu[:])
                sig = wpool.tile([P, CC], SD, tag="sig", name=f"sig_{q}_{l}")
                nc.vector.tensor_scalar(sig[:], T[:], 0.5, 0.5, ALU.mult, ALU.add)
                A = wpool.tile([P, CC], SD, tag="A", name=f"A_{q}_{l}")
                nc.gpsimd.tensor_tensor(A[:], Zu16[:], Zu16[:], ALU.mult)
                a = wpool.tile([P, CC], SD, tag="a", name=f"a_{q}_{l}")
                nc.vector.scalar_tensor_tensor(
                    a[:], Zp[:], bias_r(bl, 0), sig[:], ALU.add, ALU.mult
                )
                # silu'' = sig - s1*T
                G = wpool.tile([P, CC], SD, tag="G", name=f"G_{q}_{l}")
                nc.gpsimd.tensor_tensor(G[:], s1[:], T[:], ALU.mult)
                sil2 = wpool.tile([P, CC], SD, tag="sil2", name=f"sil2_{q}_{l}")
                nc.gpsimd.tensor_tensor(sil2[:], sig[:], G[:], ALU.subtract)
                u = wpool.tile([P, CC], SD, tag="u", name=f"u_{q}_{l}")
                if l == 1:
                    nc.gpsimd.tensor_tensor(u[:], s1[:], Zu16[:], ALU.mult)
                else:
                    nc.vector.tensor_tensor(u[:], s1[:], Zu16[:], ALU.mult)
                return s1, A, sil2, a, u

            def stage0(q):
                rb = rhs_bufs[q % NRB]
                nc.sync.dma_start(rb[0:12, :], S3[4 * q : 4 * q + 4, :, :, :])
                Z0 = pspool.tile([P, CC], F32, tag="ps", name=f"Z0_{q}")
                mm(Z0, lhsT0, rb)
                Mg = pspool.tile([P, CC], F32, tag="ps", name=f"Mg_{q}")
                mm(Mg, lhsTg, rb)
                Mu = pspool.tile([P, CC], F32, tag="ps", name=f"Mu_{q}")
                mm(Mu, lhsTu, rb)
                s1, A, sil2, a, u = elemwise(q, 0, Z0, Mu, 0)
                gm = wpool.tile([P, CC], SD, tag="gm", name=f"gm_{q}")
                nc.vector.tensor_tensor(gm[:], s1[:], Mg[:], ALU.mult)
                v = wpool.tile([P, CC], SD, tag="v", name=f"v_{q}")
                nc.gpsimd.tensor_tensor(v[:], sil2[:], A[:], ALU.mult)
                st[q] = {"a": a, "u": u, "gm": gm, "v": v}

            def stage_h(q, l):
                cs = st[q]
                Zp = pspool.tile([P, CC], F32, tag="ps", name=f"Zp_{q}_{l}")
                mm(Zp, lhsTh[l], cs["a"])
                Zu = pspool.tile([P, CC], F32, tag="ps", name=f"Zu_{q}_{l}")
                mm(Zu, lhsTh[l], cs["u"])
                Zg = pspool.tile([P, CC], F32, tag="ps", name=f"Zg_{q}_{l}")
                mm(Zg, lhsTh[l], cs["gm"])
                Zv = pspool.tile([P, CC], F32, tag="ps", name=f"Zv_{q}_{l}")
                if l == 0:
                    mm(Zv, lhsTh[l], cs["v"])
                else:
                    mm_acc(Zv, lhsTh[l], [cs["Bq"], cs["qn"]])
                s1, A, sil2, a, u = elemwise(q, l + 1, Zp, Zu, l + 1)
                gn = wpool.tile([P, CC], SD, tag="gm", name=f"g_{q}_{l}")
                nc.vector.tensor_tensor(gn[:], s1[:], Zg[:], ALU.mult)
                qn = wpool.tile([P, CC], SD, tag="q", name=f"q_{q}_{l}")
                nc.vector.tensor_tensor(qn[:], s1[:], Zv[:], ALU.mult)
                Bq = wpool.tile([P, CC], SD, tag="Bq", name=f"Bq_{q}_{l}")
                nc.gpsimd.tensor_tensor(Bq[:], sil2[:], A[:], ALU.mult)
                st[q] = {"a": a, "u": u, "gm": gn, "qn": qn, "Bq": Bq}

            def stage4(q):
                cs = st.pop(q)
                Zf = pspool.tile([NG, CC], F32, tag="ps", name=f"Zf_{q}")
                mm(Zf, lhsTf, cs["a"])
                Zuf = pspool.tile([NG, CC], F32, tag="ps", name=f"Zuf_{q}")
                mm(Zuf, lhsTf, cs["u"])
                Zgv = pspool.tile([NG, CC], F32, tag="ps", name=f"Zgv_{q}")
                mm_acc(Zgv, lhsTf, [cs["gm"], cs["Bq"], cs["qn"]])
                # bounce tile: compute writes need 32-aligned partition
                # starts; the DMA below has no such constraint.
                zc = zcpool.tile([68, CC], SD, tag="zc", name=f"zc_{q}")
                nc.scalar.activation(
                    zc[0:4, :], Zf[:], AF.Tanh, bias=bfh[0:4, :], scale=0.5
                )
                nc.scalar.activation(zc[32:36, :], Zuf[:], AF.Square)
                nc.scalar.copy(zc[64:68, :], Zgv[:])
                dq = slice(4 * q, 4 * q + 4)
                nc.sync.dma_start(
                    TfS[dq, :, :], zc[0:4, :].rearrange("p (b n) -> p b n", b=NB)
                )
                nc.sync.dma_start(
                    U2S[dq, :, :], zc[32:36, :].rearrange("p (b n) -> p b n", b=NB)
                )
                nc.sync.dma_start(
                    GVS[dq, :, :], zc[64:68, :].rearrange("p (b n) -> p b n", b=NB)
                )

            stages = [
                stage0,
                lambda q: stage_h(q, 0),
                lambda q: stage_h(q, 1),
                lambda q: stage_h(q, 2),
                stage4,
            ]
            NS = len(stages)
            for t in range(NCHUNK + NS - 1):
                for s in range(NS - 1, -1, -1):
                    q = t - s
                    if 0 <= q < NCHUNK:
                        stages[s](q)

            # ---- stage D: phi assembly + reduction ----
            Q = sgpool.tile([P, NB, NSTEP], SD, tag="dQ")
            nc.vector.tensor_tensor(Q[:], U2S[:], TfS[:], ALU.mult)
            S = sgpool.tile([P, NB, NSTEP], SD, tag="dS")
            nc.vector.tensor_tensor(S[:], GVS[:], Q[:], ALU.subtract)
            E = sgpool.tile([P, NB, NSTEP], SD, tag="dE")
            nc.gpsimd.tensor_tensor(E[:], TfS[:], TfS[:], ALU.mult)
            sp = sgpool.tile([P, NB, NSTEP], SD, tag="dsp")
            nc.vector.tensor_scalar(sp[:], E[:], -0.25, 0.25, ALU.mult, ALU.add)
            S2 = sgpool.tile([P, NB, NSTEP], SD, tag="dS2")
            nc.vector.tensor_tensor(S2[:], S[:], sp[:], ALU.mult)
            vT = sgpool.tile([P, NB, 1], F32, tag="vT")
            nc.vector.tensor_reduce(vT[:], S2[:], mybir.AxisListType.X, ALU.add)

            # ---- outputs ----
            nc.sync.dma_start(yS_d[:], sfull[:, :, NSTEP : NSTEP + 1])
            nc.sync.dma_start(yV_d[:], vT[:])

    _legalize_waits(nc)
    return nc


def _prep_host(inputs):
    rnorm = np.ascontiguousarray(np.asarray(inputs["rnorm"], dtype=np.float32))
    W0 = np.asarray(inputs["W0"], dtype=np.float32)
    b0 = np.asarray(inputs["b0"], dtype=np.float32)
    Wh = np.asarray(inputs["Wh"], dtype=np.float32)
    bh = np.asarray(inputs["bh"], dtype=np.float32)
    Wf = np.asarray(inputs["Wf"], dtype=np.float32)
    bf = np.asarray(inputs["bf"], dtype=np.float32)

    sd_np = mybir.dt.np(SD)

    # static rhs rows: t-row (period 128), ones-row
    trow = np.ones((2, CC), np.float32)
    trow[0, :] = DT * np.tile(np.arange(NSTEP, dtype=np.float32), CC // NSTEP)

    # lhsT seeds [14, P]: row 3g+s for per-group streams, 12 t, 13 ones
    lhsT0 = np.zeros((14, P), np.float32)
    lhsTg = np.zeros((14, P), np.float32)
    lhsTu = np.zeros((14, P), np.float32)
    for g in range(NG):
        cols = slice(32 * g, 32 * (g + 1))
        lhsT0[3 * g + 0, cols] = W0[:, 1]              # s coefficient
        lhsTg[3 * g + 1, cols] = W0[:, 1]              # Ds row
        lhsTu[3 * g + 2, cols] = W0[:, 1] * np.sqrt(0.5) * SIG
        lhsT0[12, cols] = W0[:, 0]                     # t row
        lhsTg[13, cols] = W0[:, 0] * DT                # ones row -> dhdt*dt

    lhsTh = np.zeros((NH, P, P), np.float32)
    for l in range(NH):
        for g in range(NG):
            blk = slice(32 * g, 32 * (g + 1))
            lhsTh[l, blk, blk] = Wh[l].T
    lhsTf = np.zeros((P, NG), np.float32)
    for g in range(NG):
        lhsTf[32 * g : 32 * (g + 1), g] = Wf[0]

    bias = np.zeros((P, 4, 2), np.float32)
    bias[:, 0, 0] = np.tile(b0, NG)
    bias[:, 0, 1] = 0.5 * bias[:, 0, 0]
    for l in range(NH):
        bias[:, l + 1, 0] = np.tile(bh[l], NG)
        bias[:, l + 1, 1] = 0.5 * bias[:, l + 1, 0]
    bfh = np.full((P, 1), 0.5 * bf[0], np.float32)

    shared = {
        "trow": trow.astype(sd_np),
        "lhsT0": lhsT0.astype(sd_np),
        "lhsTg": lhsTg.astype(sd_np),
        "lhsTu": lhsTu.astype(sd_np),
        "lhsTh": lhsTh.astype(sd_np),
        "lhsTf": lhsTf.astype(sd_np),
        "bias": bias,
        "bfh": bfh,
    }

    in_maps = []
    for core in range(NCORE):
        shard = rnorm[core * BC : (core + 1) * BC]          # [1024, 128]
        sg = np.ascontiguousarray(
            shard.reshape(NB, P, NSTEP).transpose(1, 0, 2).reshape(P, NB * NSTEP)
        )
        in_maps.append({"rn_sg": sg, **shared})
    return in_maps


last_perf = {}


def kernel(trace=False, **inputs) -> np.ndarray:
    if "nc" not in _CACHE:
        _CACHE["nc"] = _build_program()
    nc = _CACHE["nc"]
    in_maps = _prep_host(inputs)
    res = run_bass_kernel_spmd(nc, in_maps, list(range(NCORE)), trace=trace)
    last_perf["exec_time_ns"] = res.exec_time_ns
    out = np.empty((B, 2), np.float32)
    for core in range(NCORE):
        yS = res.results[core]["yS"]                        # [128, 8]
        yV = res.results[core]["yV"]                        # [128, 8]
        blk = out[core * BC : (core + 1) * BC]
        blk[:, 0] = yS.T.reshape(-1)
        blk[:, 1] = yV.T.reshape(-1)
    return out


# revision 11
# speedup vs baseline: 3.0411x; 1.0542x over previous
"""Trainium2 Bass kernel for the deep-hedging Milstein SDE loss.

Math: the reference scan has closed-form structure. With y = [s, v]:
  s_{n+1} = s_n * m_n,  m_n = 1 + MU*dt + SIG*dW_n + 0.5*SIG^2*(dW_n^2 - dt)
  v_{n+1} = v_n + dhdt*dt + dhds*(s_{n+1}-s_n) + 0.5*SIG^2*s_n^2*dW_n^2*dhdss
where (dhdt, dhds, dhdss) are derivatives of the holding MLP h(t, s) at
(t_n, s_n).  The scan collapses to:
  1. prefix-product along steps for s_n (tensor_tensor_scan)
  2. one fully-batched forward-mode jet evaluation of the MLP over all
     B*N points with 3 tangent streams
  3. a per-path reduction over steps.

Layout per core (1024 paths, path_local = b*128 + pi for partition pi,
block b):  MLP groups g = pi % 4 (q = pi // 4), so chunk q's rhs comes
from 4 CONTIGUOUS partitions S3[4q:4q+4] via one cheap DMA per chunk
(the DMA cost model charges max bytes-per-destination-partition;
single-row gathers are ~32x more expensive).  rhs rows: p = 3g+s for
the per-group streams (sN, Ds, sdW), rows 12/13 = shared static
t-row / ones-row.  Final-layer outputs bounce through a per-chunk zc
tile (compute writes need 32-aligned partition starts) and DMA to
sgrid tiles at partitions [4q:4q+4] - same path order as stage A.

Engine notes (CoreSim cost model + walrus ISA constraints):
  - GPSIMD (Pool) cannot read PSUM and only runs TensorTensor; it gets
    all-SBUF f16 multiplies (A, G, sil2, Bq, u, v) at a flat 878ns.
  - ACT evacuates each layer's Zu once (Zu16, scalar.copy) feeding
    Pool's A and u; silu'' = sig - s1*T avoids TensorScalarPtr on Pool.
  - v = Bq + q is folded into the next layer's Zv matmul as two
    accumulating matmuls (PE has slack).
  - Chunks are software-pipelined with a 5-stage skew (L0, h0, h1, h2,
    final) so in-order engine queues interleave 5 independent chunks.
"""

import numpy as np

import concourse.bass as bass
import concourse.mybir as mybir
from concourse import tile
from concourse.bass_utils import run_bass_kernel_spmd


# problem constants (hardcoded per spec)
B = 8192
NSTEP = 128
NCORE = 8
BC = B // NCORE          # 1024 paths per core
P = 128                  # partitions
NB = BC // P             # 8 path blocks
WIDTH = 32
NG = 4                   # feature groups on partitions
NH = 3                   # hidden layers
NQ = 32                  # within-group path index == chunk index
CC = NB * NSTEP          # 1024 point-columns per chunk
NCHUNK = NQ              # 32
NRB = 3                  # rhs buffers
T0, T1 = 0.0, 1.0
MU, SIG = 1.0, 1.0
DT = (T1 - T0) / NSTEP
SQDT = float(np.sqrt(DT))

F32 = mybir.dt.float32
AF = mybir.ActivationFunctionType
ALU = mybir.AluOpType

SD = mybir.dt.float16

_CACHE = {}


def _legalize_waits(nc):
    """Split long on_wait lists into standalone single-wait NoOps.

    This walrus rejects instructions whose sync_info carries more waits
    than the ISA encoding holds; spill the excess onto NoOps on the same
    engine queue, which execute in order before the real instruction.
    """
    ctr = 0
    for bb in nc.main_func.blocks:
        out = []
        for ins in bb.instructions:
            si = ins.sync_info
            if si is not None and si.on_wait:
                limit = 1
                waits = list(si.on_wait)
                if len(waits) > limit:
                    spill, keep = waits[:-limit], waits[-limit:]
                    for w in spill:
                        ctr += 1
                        nop = mybir.InstNoOp(
                            name=f"waitnop_{ctr}", ins=[], outs=[]
                        )
                        nop.engine = ins.engine
                        nop.sync_info = mybir.SyncInfo(on_wait=[w], on_update=[])
                        out.append(nop)
                    si.on_wait = keep
            out.append(ins)
        bb.instructions = out


def _build_program():
    nc = bass.Bass()

    rn_d = nc.declare_dram_parameter("rn_sg", [P, NB * NSTEP], F32, isOutput=False)
    trow_d = nc.declare_dram_parameter("trow", [2, CC], SD, isOutput=False)
    lhsT0_d = nc.declare_dram_parameter("lhsT0", [14, P], SD, isOutput=False)
    lhsTg_d = nc.declare_dram_parameter("lhsTg", [14, P], SD, isOutput=False)
    lhsTu_d = nc.declare_dram_parameter("lhsTu", [14, P], SD, isOutput=False)
    lhsTh_d = nc.declare_dram_parameter("lhsTh", [NH, P, P], SD, isOutput=False)
    lhsTf_d = nc.declare_dram_parameter("lhsTf", [P, NG], SD, isOutput=False)
    bias_d = nc.declare_dram_parameter("bias", [P, 4, 2], F32, isOutput=False)
    bfh_d = nc.declare_dram_parameter("bfh", [P, 1], F32, isOutput=False)
    yS_d = nc.declare_dram_parameter("yS", [P, NB], F32, isOutput=True)
    yV_d = nc.declare_dram_parameter("yV", [P, NB], F32, isOutput=True)

    HC = CC // 2

    with tile.TileContext(nc) as tc:
        with (
            tc.tile_pool(name="const", bufs=1) as cpool,
            tc.tile_pool(name="sg", bufs=1) as sgpool,
            tc.tile_pool(name="work", bufs=5) as wpool,
            tc.tile_pool(name="zcp", bufs=3) as zcpool,
            tc.tile_pool(name="psum", bufs=4, space="PSUM") as pspool,
        ):
            # ---- load constants ----
            lhsT0 = cpool.tile([14, P], SD, tag="lhsT0")
            lhsTg = cpool.tile([14, P], SD, tag="lhsTg")
            lhsTu = cpool.tile([14, P], SD, tag="lhsTu")
            lhsTh = [
                cpool.tile([P, P], SD, tag=f"lhsTh{l}", name=f"lhsTh{l}")
                for l in range(NH)
            ]
            lhsTf = cpool.tile([P, NG], SD, tag="lhsTf")
            bias = cpool.tile([P, 4, 2], F32, tag="bias")
            bfh = cpool.tile([P, 1], F32, tag="bfh")
            nc.sync.dma_start(lhsT0[:], lhsT0_d[:])
            nc.sync.dma_start(lhsTg[:], lhsTg_d[:])
            nc.sync.dma_start(lhsTu[:], lhsTu_d[:])
            for l in range(NH):
                nc.sync.dma_start(lhsTh[l][:], lhsTh_d[l])
            nc.sync.dma_start(lhsTf[:], lhsTf_d[:])
            nc.sync.dma_start(bias[:], bias_d[:])
            nc.sync.dma_start(bfh[:], bfh_d[:])

            def bias_r(l, h):
                return bias[:, l, h : h + 1]

            # rhs chunk buffers; static rows 12 (t) / 13 (ones) filled once
            rhs_bufs = [
                cpool.tile([14, CC], SD, tag=f"rhs{k}", name=f"rhs{k}")
                for k in range(NRB)
            ]
            for k in range(NRB):
                nc.sync.dma_start(rhs_bufs[k][12:14, :], trow_d[:])

            # ---- stage A: sgrid GBM math ----
            rs = sgpool.tile([P, NB, NSTEP], F32, tag="rs")
            nc.sync.dma_start(rs[:], rn_d[:].rearrange("p (b n) -> p b n", b=NB))
            dW = sgpool.tile([P, NB, NSTEP], F32, tag="dW")
            nc.vector.tensor_scalar_mul(dW[:], rs[:], SQDT)
            m = sgpool.tile([P, NB, NSTEP], F32, tag="m")
            nc.vector.tensor_mul(rs[:], dW[:], dW[:])
            nc.vector.scalar_tensor_tensor(
                m[:], rs[:], 0.5 * SIG * SIG, dW[:], ALU.mult, ALU.add
            )
            c0 = 1.0 + MU * DT - 0.5 * SIG * SIG * DT
            nc.vector.tensor_scalar_add(m[:], m[:], c0)

            sfull = sgpool.tile([P, NB, NSTEP + 1], F32, tag="sfull")
            nc.vector.memset(sfull[:, :, 0:1], 1.0)
            for b in range(NB):
                nc.vector.tensor_tensor_scan(
                    sfull[:, b, 1 : NSTEP + 1],
                    m[:, b, :],
                    m[:, b, :],
                    1.0,
                    ALU.mult,
                    ALU.bypass,
                )
            sN = sfull[:, :, 0:NSTEP]

            # S3[:, 0]=sN, [:, 1]=Ds=(m-1)*sN, [:, 2]=sdW=sN*dW   (f16)
            S3 = sgpool.tile([P, 3, NB, NSTEP], SD, tag="S3")
            nc.scalar.copy(S3[:, 0], sN)
            nc.vector.scalar_tensor_tensor(
                S3[:, 1], m[:], 1.0, sN, ALU.subtract, ALU.mult
            )
            nc.gpsimd.tensor_tensor(S3[:, 2], sN, dW[:], ALU.mult)

            # final-output sgrid tiles, filled per chunk at [4q:4q+4]
            TfS = sgpool.tile([P, NB, NSTEP], SD, tag="TfS")
            U2S = sgpool.tile([P, NB, NSTEP], SD, tag="U2S")
            GVS = sgpool.tile([P, NB, NSTEP], SD, tag="GVS")

            def mm(out, lhsT, rhs):
                # PSUM banks are 2KB; a single matmul output must stay in
                # one bank, so emit one matmul per 512-col half.
                for h in range(2):
                    cs = slice(h * HC, (h + 1) * HC)
                    nc.tensor.matmul(
                        out[:, cs], lhsT[:], rhs[:, cs], start=True, stop=True
                    )

            def mm_acc(out, lhsT, rhss):
                for h in range(2):
                    cs = slice(h * HC, (h + 1) * HC)
                    for i, r in enumerate(rhss):
                        nc.tensor.matmul(
                            out[:, cs], lhsT[:], r[:, cs],
                            start=(i == 0), stop=(i == len(rhss) - 1),
                        )

            # ---- software-pipelined chunk loop (5-stage skew) ----
            st = {}  # q -> carried stream tiles

            def elemwise_act(q, l, Zp, Zu, bl):
                s1 = wpool.tile([P, CC], SD, tag="s1", name=f"s1_{q}_{l}")
                nc.scalar.activation(
                    s1[:], Zp[:], AF.Derivative_silu, bias=bias_r(bl, 0)
                )
                T = wpool.tile([P, CC], SD, tag="T", name=f"T_{q}_{l}")
                nc.scalar.activation(
                    T[:], Zp[:], AF.Tanh, bias=bias_r(bl, 1), scale=0.5
                )
                Zu16 = wpool.tile([P, CC], SD, tag="Zu16", name=f"Zu16_{q}_{l}")
                nc.scalar.copy(Zu16[:], Zu[:])
                return s1, T, Zu16

            def elemwise_rest(q, l, Zp, s1, T, Zu16, bl):
                sig = wpool.tile([P, CC], SD, tag="sig", name=f"sig_{q}_{l}")
                nc.vector.tensor_scalar(sig[:], T[:], 0.5, 0.5, ALU.mult, ALU.add)
                a = wpool.tile([P, CC], SD, tag="a", name=f"a_{q}_{l}")
                nc.vector.scalar_tensor_tensor(
                    a[:], Zp[:], bias_r(bl, 0), sig[:], ALU.add, ALU.mult
                )
                A = wpool.tile([P, CC], SD, tag="A", name=f"A_{q}_{l}")
                nc.gpsimd.tensor_tensor(A[:], Zu16[:], Zu16[:], ALU.mult)
                # silu'' = sig - s1*T
                G = wpool.tile([P, CC], SD, tag="G", name=f"G_{q}_{l}")
                nc.gpsimd.tensor_tensor(G[:], s1[:], T[:], ALU.mult)
                sil2 = wpool.tile([P, CC], SD, tag="sil2", name=f"sil2_{q}_{l}")
                nc.gpsimd.tensor_tensor(sil2[:], sig[:], G[:], ALU.subtract)
                u = wpool.tile([P, CC], SD, tag="u", name=f"u_{q}_{l}")
                if l in (1, 2):
                    nc.gpsimd.tensor_tensor(u[:], s1[:], Zu16[:], ALU.mult)
                else:
                    nc.vector.tensor_tensor(u[:], s1[:], Zu16[:], ALU.mult)
                return A, sil2, a, u

            def prefetch(q):
                rb = rhs_bufs[q % NRB]
                nc.sync.dma_start(rb[0:12, :], S3[4 * q : 4 * q + 4, :, :, :])

            def stage0(q):
                rb = rhs_bufs[q % NRB]
                Z0 = pspool.tile([P, CC], F32, tag="ps", name=f"Z0_{q}")
                mm(Z0, lhsT0, rb)
                Mg = pspool.tile([P, CC], F32, tag="ps", name=f"Mg_{q}")
                mm(Mg, lhsTg, rb)
                Mu = pspool.tile([P, CC], F32, tag="ps", name=f"Mu_{q}")
                mm(Mu, lhsTu, rb)
                s1, T, Zu16 = elemwise_act(q, 0, Z0, Mu, 0)
                gm = wpool.tile([P, CC], SD, tag="gm", name=f"gm_{q}")
                nc.vector.tensor_tensor(gm[:], s1[:], Mg[:], ALU.mult)
                A, sil2, a, u = elemwise_rest(q, 0, Z0, s1, T, Zu16, 0)
                v = wpool.tile([P, CC], SD, tag="v", name=f"v_{q}")
                nc.gpsimd.tensor_tensor(v[:], sil2[:], A[:], ALU.mult)
                st[q] = {"a": a, "u": u, "gm": gm, "v": v}

            def stage_h(q, l):
                cs = st[q]
                Zp = pspool.tile([P, CC], F32, tag="ps", name=f"Zp_{q}_{l}")
                mm(Zp, lhsTh[l], cs["a"])
                Zu = pspool.tile([P, CC], F32, tag="ps", name=f"Zu_{q}_{l}")
                mm(Zu, lhsTh[l], cs["u"])
                Zg = pspool.tile([P, CC], F32, tag="ps", name=f"Zg_{q}_{l}")
                mm(Zg, lhsTh[l], cs["gm"])
                Zv = pspool.tile([P, CC], F32, tag="ps", name=f"Zv_{q}_{l}")
                if l == 0:
                    mm(Zv, lhsTh[l], cs["v"])
                else:
                    mm_acc(Zv, lhsTh[l], [cs["Bq"], cs["qn"]])
                s1, T, Zu16 = elemwise_act(q, l + 1, Zp, Zu, l + 1)
                gn = wpool.tile([P, CC], SD, tag="gm", name=f"g_{q}_{l}")
                nc.vector.tensor_tensor(gn[:], s1[:], Zg[:], ALU.mult)
                qn = wpool.tile([P, CC], SD, tag="q", name=f"q_{q}_{l}")
                nc.vector.tensor_tensor(qn[:], s1[:], Zv[:], ALU.mult)
                A, sil2, a, u = elemwise_rest(q, l + 1, Zp, s1, T, Zu16, l + 1)
                Bq = wpool.tile([P, CC], SD, tag="Bq", name=f"Bq_{q}_{l}")
                nc.gpsimd.tensor_tensor(Bq[:], sil2[:], A[:], ALU.mult)
                st[q] = {"a": a, "u": u, "gm": gn, "qn": qn, "Bq": Bq}

            def stage4(q):
                cs = st.pop(q)
                Zf = pspool.tile([NG, CC], F32, tag="ps", name=f"Zf_{q}")
                mm(Zf, lhsTf, cs["a"])
                Zuf = pspool.tile([NG, CC], F32, tag="ps", name=f"Zuf_{q}")
                mm(Zuf, lhsTf, cs["u"])
                Zgv = pspool.tile([NG, CC], F32, tag="ps", name=f"Zgv_{q}")
                mm_acc(Zgv, lhsTf, [cs["gm"], cs["Bq"], cs["qn"]])
                # bounce tile: compute writes need 32-aligned partition
                # starts; the DMA below has no such constraint.
                zc = zcpool.tile([68, CC], SD, tag="zc", name=f"zc_{q}")
                nc.scalar.activation(
                    zc[0:4, :], Zf[:], AF.Tanh, bias=bfh[0:4, :], scale=0.5
                )
                nc.scalar.activation(zc[32:36, :], Zuf[:], AF.Square)
                nc.scalar.copy(zc[64:68, :], Zgv[:])
                dq = slice(4 * q, 4 * q + 4)
                nc.sync.dma_start(
                    TfS[dq, :, :], zc[0:4, :].rearrange("p (b n) -> p b n", b=NB)
                )
                nc.sync.dma_start(
                    U2S[dq, :, :], zc[32:36, :].rearrange("p (b n) -> p b n", b=NB)
                )
                nc.sync.dma_start(
                    GVS[dq, :, :], zc[64:68, :].rearrange("p (b n) -> p b n", b=NB)
                )

            stages = [
                prefetch,
                stage0,
                lambda q: stage_h(q, 0),
                lambda q: stage_h(q, 1),
                lambda q: stage_h(q, 2),
                stage4,
            ]
            NS = len(stages)
            for t in range(NCHUNK + NS - 1):
                for s in range(NS - 1, -1, -1):
                    q = t - s
                    if 0 <= q < NCHUNK:
                        stages[s](q)

            # ---- stage D: phi assembly + reduction ----
            Q = sgpool.tile([P, NB, NSTEP], SD, tag="dQ")
            nc.vector.tensor_tensor(Q[:], U2S[:], TfS[:], ALU.mult)
            S = sgpool.tile([P, NB, NSTEP], SD, tag="dS")
            nc.vector.tensor_tensor(S[:], GVS[:], Q[:], ALU.subtract)
            E = sgpool.tile([P, NB, NSTEP], SD, tag="dE")
            nc.gpsimd.tensor_tensor(E[:], TfS[:], TfS[:], ALU.mult)
            sp = sgpool.tile([P, NB, NSTEP], SD, tag="dsp")
            nc.vector.tensor_scalar(sp[:], E[:], -0.25, 0.25, ALU.mult, ALU.add)
            S2 = sgpool.tile([P, NB, NSTEP], SD, tag="dS2")
            nc.vector.tensor_tensor(S2[:], S[:], sp[:], ALU.mult)
            vT = sgpool.tile([P, NB, 1], F32, tag="vT")
            nc.vector.tensor_reduce(vT[:], S2[:], mybir.AxisListType.X, ALU.add)

            # ---- outputs ----
            nc.sync.dma_start(yS_d[:], sfull[:, :, NSTEP : NSTEP + 1])
            nc.sync.dma_start(yV_d[:], vT[:])

    _legalize_waits(nc)
    return nc


def _prep_host(inputs):
    rnorm = np.ascontiguousarray(np.asarray(inputs["rnorm"], dtype=np.float32))
    W0 = np.asarray(inputs["W0"], dtype=np.float32)
    b0 = np.asarray(inputs["b0"], dtype=np.float32)
    Wh = np.asarray(inputs["Wh"], dtype=np.float32)
    bh = np.asarray(inputs["bh"], dtype=np.float32)
    Wf = np.asarray(inputs["Wf"], dtype=np.float32)
    bf = np.asarray(inputs["bf"], dtype=np.float32)

    sd_np = mybir.dt.np(SD)

    # static rhs rows: t-row (period 128), ones-row
    trow = np.ones((2, CC), np.float32)
    trow[0, :] = DT * np.tile(np.arange(NSTEP, dtype=np.float32), CC // NSTEP)

    # lhsT seeds [14, P]: row 3g+s for per-group streams, 12 t, 13 ones
    lhsT0 = np.zeros((14, P), np.float32)
    lhsTg = np.zeros((14, P), np.float32)
    lhsTu = np.zeros((14, P), np.float32)
    for g in range(NG):
        cols = slice(32 * g, 32 * (g + 1))
        lhsT0[3 * g + 0, cols] = W0[:, 1]              # s coefficient
        lhsTg[3 * g + 1, cols] = W0[:, 1]              # Ds row
        lhsTu[3 * g + 2, cols] = W0[:, 1] * np.sqrt(0.5) * SIG
        lhsT0[12, cols] = W0[:, 0]                     # t row
        lhsTg[13, cols] = W0[:, 0] * DT                # ones row -> dhdt*dt

    lhsTh = np.zeros((NH, P, P), np.float32)
    for l in range(NH):
        for g in range(NG):
            blk = slice(32 * g, 32 * (g + 1))
            lhsTh[l, blk, blk] = Wh[l].T
    lhsTf = np.zeros((P, NG), np.float32)
    for g in range(NG):
        lhsTf[32 * g : 32 * (g + 1), g] = Wf[0]

    bias = np.zeros((P, 4, 2), np.float32)
    bias[:, 0, 0] = np.tile(b0, NG)
    bias[:, 0, 1] = 0.5 * bias[:, 0, 0]
    for l in range(NH):
        bias[:, l + 1, 0] = np.tile(bh[l], NG)
        bias[:, l + 1, 1] = 0.5 * bias[:, l + 1, 0]
    bfh = np.full((P, 1), 0.5 * bf[0], np.float32)

    shared = {
        "trow": trow.astype(sd_np),
        "lhsT0": lhsT0.astype(sd_np),
        "lhsTg": lhsTg.astype(sd_np),
        "lhsTu": lhsTu.astype(sd_np),
        "lhsTh": lhsTh.astype(sd_np),
        "lhsTf": lhsTf.astype(sd_np),
        "bias": bias,
        "bfh": bfh,
    }

    in_maps = []
    for core in range(NCORE):
        shard = rnorm[core * BC : (core + 1) * BC]          # [1024, 128]
        sg = np.ascontiguousarray(
            shard.reshape(NB, P, NSTEP).transpose(1, 0, 2).reshape(P, NB * NSTEP)
        )
        in_maps.append({"rn_sg": sg, **shared})
    return in_maps


last_perf = {}


def kernel(trace=False, **inputs) -> np.ndarray:
    if "nc" not in _CACHE:
        _CACHE["nc"] = _build_program()
    nc = _CACHE["nc"]
    in_maps = _prep_host(inputs)
    res = run_bass_kernel_spmd(nc, in_maps, list(range(NCORE)), trace=trace)
    last_perf["exec_time_ns"] = res.exec_time_ns
    out = np.empty((B, 2), np.float32)
    for core in range(NCORE):
        yS = res.results[core]["yS"]                        # [128, 8]
        yV = res.results[core]["yV"]                        # [128, 8]
        blk = out[core * BC : (core + 1) * BC]
        blk[:, 0] = yS.T.reshape(-1)
        blk[:, 1] = yV.T.reshape(-1)
    return out


# revision 12
# speedup vs baseline: 3.3249x; 1.0933x over previous
"""Trainium2 Bass kernel for the deep-hedging Milstein SDE loss.

Math: the reference scan has closed-form structure. With y = [s, v]:
  s_{n+1} = s_n * m_n,  m_n = 1 + MU*dt + SIG*dW_n + 0.5*SIG^2*(dW_n^2 - dt)
  v_{n+1} = v_n + dhdt*dt + dhds*(s_{n+1}-s_n) + 0.5*SIG^2*s_n^2*dW_n^2*dhdss
where (dhdt, dhds, dhdss) are derivatives of the holding MLP h(t, s) at
(t_n, s_n).  The scan collapses to:
  1. prefix-product along steps for s_n (tensor_tensor_scan)
  2. one fully-batched forward-mode jet evaluation of the MLP over all
     B*N points with 3 tangent streams
  3. a per-path reduction over steps.

Layout per core (1024 paths, path_local = b*128 + pi for partition pi,
block b):  MLP groups g = pi % 4 (q = pi // 4), so chunk q's rhs comes
from 4 CONTIGUOUS partitions S3[4q:4q+4] via one cheap DMA per chunk
(the DMA cost model charges max bytes-per-destination-partition;
single-row gathers are ~32x more expensive).  rhs rows: p = 3g+s for
the per-group streams (sN, Ds, sdW), rows 12/13 = shared static
t-row / ones-row.  Final-layer outputs bounce through a per-chunk zc
tile (compute writes need 32-aligned partition starts) and DMA to
sgrid tiles at partitions [4q:4q+4] - same path order as stage A.

Engine notes (CoreSim cost model + walrus ISA constraints):
  - GPSIMD (Pool) cannot read PSUM and only runs TensorTensor; it gets
    all-SBUF f16 multiplies (A, G, sil2, Bq, u, v) at a flat 878ns.
  - ACT evacuates each layer's Zu once (Zu16, scalar.copy) feeding
    Pool's A and u; silu'' = sig - s1*T avoids TensorScalarPtr on Pool.
  - v = Bq + q is folded into the next layer's Zv matmul as two
    accumulating matmuls (PE has slack).
  - Chunks are software-pipelined with a 5-stage skew (L0, h0, h1, h2,
    final) so in-order engine queues interleave 5 independent chunks.
"""

import numpy as np

import concourse.bass as bass
import concourse.mybir as mybir
from concourse import tile
from concourse.bass_utils import run_bass_kernel_spmd


# problem constants (hardcoded per spec)
B = 8192
NSTEP = 128
NCORE = 8
BC = B // NCORE          # 1024 paths per core
P = 128                  # partitions
NB = BC // P             # 8 path blocks
WIDTH = 32
NG = 4                   # feature groups on partitions
NH = 3                   # hidden layers
NQ = 32                  # within-group path index == chunk index
CC = NB * NSTEP          # 1024 point-columns per chunk
NCHUNK = NQ              # 32
NRB = 3                  # rhs buffers
T0, T1 = 0.0, 1.0
MU, SIG = 1.0, 1.0
DT = (T1 - T0) / NSTEP
SQDT = float(np.sqrt(DT))

F32 = mybir.dt.float32
AF = mybir.ActivationFunctionType
ALU = mybir.AluOpType

SD = mybir.dt.float16

_CACHE = {}


def _legalize_waits(nc):
    """Split long on_wait lists into standalone single-wait NoOps.

    This walrus rejects instructions whose sync_info carries more waits
    than the ISA encoding holds; spill the excess onto NoOps on the same
    engine queue, which execute in order before the real instruction.
    """
    ctr = 0
    for bb in nc.main_func.blocks:
        out = []
        for ins in bb.instructions:
            si = ins.sync_info
            if si is not None and si.on_wait:
                limit = 1
                waits = list(si.on_wait)
                if len(waits) > limit:
                    spill, keep = waits[:-limit], waits[-limit:]
                    for w in spill:
                        ctr += 1
                        nop = mybir.InstNoOp(
                            name=f"waitnop_{ctr}", ins=[], outs=[]
                        )
                        nop.engine = ins.engine
                        nop.sync_info = mybir.SyncInfo(on_wait=[w], on_update=[])
                        out.append(nop)
                    si.on_wait = keep
            out.append(ins)
        bb.instructions = out


def _build_program():
    nc = bass.Bass()

    rn_d = nc.declare_dram_parameter("rn_sg", [P, NB * NSTEP], F32, isOutput=False)
    trow_d = nc.declare_dram_parameter("trow", [2, CC], SD, isOutput=False)
    lhsT0_d = nc.declare_dram_parameter("lhsT0", [14, P], SD, isOutput=False)
    lhsTg_d = nc.declare_dram_parameter("lhsTg", [14, P], SD, isOutput=False)
    lhsTu_d = nc.declare_dram_parameter("lhsTu", [14, P], SD, isOutput=False)
    lhsTh_d = nc.declare_dram_parameter("lhsTh", [NH, P, P], SD, isOutput=False)
    lhsTf_d = nc.declare_dram_parameter("lhsTf", [P, NG], SD, isOutput=False)
    bias_d = nc.declare_dram_parameter("bias", [P, 4, 2], F32, isOutput=False)
    bfh_d = nc.declare_dram_parameter("bfh", [P, 1], F32, isOutput=False)
    yS_d = nc.declare_dram_parameter("yS", [P, NB], F32, isOutput=True)
    yV_d = nc.declare_dram_parameter("yV", [P, NB], F32, isOutput=True)

    HC = CC // 2

    with tile.TileContext(nc) as tc:
        with (
            tc.tile_pool(name="const", bufs=1) as cpool,
            tc.tile_pool(name="sg", bufs=1) as sgpool,
            tc.tile_pool(name="work", bufs=5) as wpool,
            tc.tile_pool(name="zcp", bufs=3) as zcpool,
            tc.tile_pool(name="psum", bufs=4, space="PSUM") as pspool,
        ):
            # ---- load constants ----
            lhsT0 = cpool.tile([14, P], SD, tag="lhsT0")
            lhsTg = cpool.tile([14, P], SD, tag="lhsTg")
            lhsTu = cpool.tile([14, P], SD, tag="lhsTu")
            lhsTh = [
                cpool.tile([P, P], SD, tag=f"lhsTh{l}", name=f"lhsTh{l}")
                for l in range(NH)
            ]
            lhsTf = cpool.tile([P, NG], SD, tag="lhsTf")
            bias = cpool.tile([P, 4, 2], F32, tag="bias")
            bfh = cpool.tile([P, 1], F32, tag="bfh")
            nc.sync.dma_start(lhsT0[:], lhsT0_d[:])
            nc.sync.dma_start(lhsTg[:], lhsTg_d[:])
            nc.sync.dma_start(lhsTu[:], lhsTu_d[:])
            for l in range(NH):
                nc.sync.dma_start(lhsTh[l][:], lhsTh_d[l])
            nc.sync.dma_start(lhsTf[:], lhsTf_d[:])
            nc.sync.dma_start(bias[:], bias_d[:])
            nc.sync.dma_start(bfh[:], bfh_d[:])

            def bias_r(l, h):
                return bias[:, l, h : h + 1]

            # rhs chunk buffers; static rows 12 (t) / 13 (ones) filled once
            rhs_bufs = [
                cpool.tile([14, CC], SD, tag=f"rhs{k}", name=f"rhs{k}")
                for k in range(NRB)
            ]
            for k in range(NRB):
                nc.sync.dma_start(rhs_bufs[k][12:14, :], trow_d[:])

            # ---- stage A: sgrid GBM math ----
            rs = sgpool.tile([P, NB, NSTEP], F32, tag="rs")
            nc.sync.dma_start(rs[:], rn_d[:].rearrange("p (b n) -> p b n", b=NB))
            dW = sgpool.tile([P, NB, NSTEP], F32, tag="dW")
            nc.vector.tensor_scalar_mul(dW[:], rs[:], SQDT)
            m = sgpool.tile([P, NB, NSTEP], F32, tag="m")
            nc.vector.tensor_mul(rs[:], dW[:], dW[:])
            nc.vector.scalar_tensor_tensor(
                m[:], rs[:], 0.5 * SIG * SIG, dW[:], ALU.mult, ALU.add
            )
            c0 = 1.0 + MU * DT - 0.5 * SIG * SIG * DT
            nc.vector.tensor_scalar_add(m[:], m[:], c0)

            sfull = sgpool.tile([P, NB, NSTEP + 1], F32, tag="sfull")
            nc.vector.memset(sfull[:, :, 0:1], 1.0)
            for b in range(NB):
                nc.vector.tensor_tensor_scan(
                    sfull[:, b, 1 : NSTEP + 1],
                    m[:, b, :],
                    m[:, b, :],
                    1.0,
                    ALU.mult,
                    ALU.bypass,
                )
            sN = sfull[:, :, 0:NSTEP]

            # S3[:, 0]=sN, [:, 1]=Ds=(m-1)*sN, [:, 2]=sdW=sN*dW   (f16)
            S3 = sgpool.tile([P, 3, NB, NSTEP], SD, tag="S3")
            nc.scalar.copy(S3[:, 0], sN)
            nc.vector.scalar_tensor_tensor(
                S3[:, 1], m[:], 1.0, sN, ALU.subtract, ALU.mult
            )
            nc.gpsimd.tensor_tensor(S3[:, 2], sN, dW[:], ALU.mult)

            # final-output sgrid tiles, filled per chunk at [4q:4q+4]
            TfS = sgpool.tile([P, NB, NSTEP], SD, tag="TfS")
            U2S = sgpool.tile([P, NB, NSTEP], SD, tag="U2S")
            GVS = sgpool.tile([P, NB, NSTEP], SD, tag="GVS")

            def mm(out, lhsT, rhs):
                # PSUM banks are 2KB; a single matmul output must stay in
                # one bank, so emit one matmul per 512-col half.
                for h in range(2):
                    cs = slice(h * HC, (h + 1) * HC)
                    nc.tensor.matmul(
                        out[:, cs], lhsT[:], rhs[:, cs], start=True, stop=True
                    )

            def mm_acc(out, lhsT, rhss):
                for h in range(2):
                    cs = slice(h * HC, (h + 1) * HC)
                    for i, r in enumerate(rhss):
                        nc.tensor.matmul(
                            out[:, cs], lhsT[:], r[:, cs],
                            start=(i == 0), stop=(i == len(rhss) - 1),
                        )

            # ---- software-pipelined chunk loop (5-stage skew) ----
            st = {}  # q -> carried stream tiles

            def elemwise_act(q, l, Zp, Zu, bl):
                s1 = wpool.tile([P, CC], SD, tag="s1", name=f"s1_{q}_{l}")
                nc.scalar.activation(
                    s1[:], Zp[:], AF.Derivative_silu, bias=bias_r(bl, 0)
                )
                T = wpool.tile([P, CC], SD, tag="T", name=f"T_{q}_{l}")
                nc.scalar.activation(
                    T[:], Zp[:], AF.Tanh, bias=bias_r(bl, 1), scale=0.5
                )
                Zu16 = wpool.tile([P, CC], SD, tag="Zu16", name=f"Zu16_{q}_{l}")
                if l == 2:
                    nc.vector.tensor_copy(Zu16[:], Zu[:])
                else:
                    nc.scalar.copy(Zu16[:], Zu[:])
                return s1, T, Zu16

            def elemwise_rest(q, l, Zp, s1, T, Zu16, bl):
                sig = wpool.tile([P, CC], SD, tag="sig", name=f"sig_{q}_{l}")
                nc.vector.tensor_scalar(sig[:], T[:], 0.5, 0.5, ALU.mult, ALU.add)
                a = wpool.tile([P, CC], SD, tag="a", name=f"a_{q}_{l}")
                nc.vector.scalar_tensor_tensor(
                    a[:], Zp[:], bias_r(bl, 0), sig[:], ALU.add, ALU.mult
                )
                A = wpool.tile([P, CC], SD, tag="A", name=f"A_{q}_{l}")
                nc.gpsimd.tensor_tensor(A[:], Zu16[:], Zu16[:], ALU.mult)
                # silu'' = sig - s1*T
                G = wpool.tile([P, CC], SD, tag="G", name=f"G_{q}_{l}")
                nc.gpsimd.tensor_tensor(G[:], s1[:], T[:], ALU.mult)
                sil2 = wpool.tile([P, CC], SD, tag="sil2", name=f"sil2_{q}_{l}")
                nc.gpsimd.tensor_tensor(sil2[:], sig[:], G[:], ALU.subtract)
                u = wpool.tile([P, CC], SD, tag="u", name=f"u_{q}_{l}")
                nc.vector.tensor_tensor(u[:], s1[:], Zu16[:], ALU.mult)
                return A, sil2, a, u

            def prefetch(q):
                rb = rhs_bufs[q % NRB]
                nc.sync.dma_start(rb[0:12, :], S3[4 * q : 4 * q + 4, :, :, :])

            def stage0(q):
                rb = rhs_bufs[q % NRB]
                Z0 = pspool.tile([P, CC], F32, tag="ps", name=f"Z0_{q}")
                mm(Z0, lhsT0, rb)
                Mg = pspool.tile([P, CC], F32, tag="ps", name=f"Mg_{q}")
                mm(Mg, lhsTg, rb)
                Mu = pspool.tile([P, CC], F32, tag="ps", name=f"Mu_{q}")
                mm(Mu, lhsTu, rb)
                s1, T, Zu16 = elemwise_act(q, 0, Z0, Mu, 0)
                gm = wpool.tile([P, CC], SD, tag="gm", name=f"gm_{q}")
                nc.vector.tensor_tensor(gm[:], s1[:], Mg[:], ALU.mult)
                A, sil2, a, u = elemwise_rest(q, 0, Z0, s1, T, Zu16, 0)
                v = wpool.tile([P, CC], SD, tag="Bq", name=f"v_{q}")
                nc.gpsimd.tensor_tensor(v[:], sil2[:], A[:], ALU.mult)
                # w = g + v merged stream, carried as the pair (wq, wB)
                st[q] = {"a": a, "u": u, "wq": gm, "wB": v}

            def stage_h(q, l):
                cs = st[q]
                Zp = pspool.tile([P, CC], F32, tag="ps", name=f"Zp_{q}_{l}")
                mm(Zp, lhsTh[l], cs["a"])
                Zu = pspool.tile([P, CC], F32, tag="ps", name=f"Zu_{q}_{l}")
                mm(Zu, lhsTh[l], cs["u"])
                Zw = pspool.tile([P, CC], F32, tag="ps", name=f"Zw_{q}_{l}")
                mm_acc(Zw, lhsTh[l], [cs["wq"], cs["wB"]])
                s1, T, Zu16 = elemwise_act(q, l + 1, Zp, Zu, l + 1)
                qw = wpool.tile([P, CC], SD, tag="q", name=f"qw_{q}_{l}")
                nc.vector.tensor_tensor(qw[:], s1[:], Zw[:], ALU.mult)
                A, sil2, a, u = elemwise_rest(q, l + 1, Zp, s1, T, Zu16, l + 1)
                Bq = wpool.tile([P, CC], SD, tag="Bq", name=f"Bq_{q}_{l}")
                nc.gpsimd.tensor_tensor(Bq[:], sil2[:], A[:], ALU.mult)
                st[q] = {"a": a, "u": u, "wq": qw, "wB": Bq}

            def stage4(q):
                cs = st.pop(q)
                Zf = pspool.tile([NG, CC], F32, tag="ps", name=f"Zf_{q}")
                mm(Zf, lhsTf, cs["a"])
                Zuf = pspool.tile([NG, CC], F32, tag="ps", name=f"Zuf_{q}")
                mm(Zuf, lhsTf, cs["u"])
                Zgv = pspool.tile([NG, CC], F32, tag="ps", name=f"Zgv_{q}")
                mm_acc(Zgv, lhsTf, [cs["wq"], cs["wB"]])
                # bounce tile: compute writes need 32-aligned partition
                # starts; the DMA below has no such constraint.
                zc = zcpool.tile([68, CC], SD, tag="zc", name=f"zc_{q}")
                nc.scalar.activation(
                    zc[0:4, :], Zf[:], AF.Tanh, bias=bfh[0:4, :], scale=0.5
                )
                nc.scalar.activation(zc[32:36, :], Zuf[:], AF.Square)
                nc.scalar.copy(zc[64:68, :], Zgv[:])
                dq = slice(4 * q, 4 * q + 4)
                nc.sync.dma_start(
                    TfS[dq, :, :], zc[0:4, :].rearrange("p (b n) -> p b n", b=NB)
                )
                nc.sync.dma_start(
                    U2S[dq, :, :], zc[32:36, :].rearrange("p (b n) -> p b n", b=NB)
                )
                nc.sync.dma_start(
                    GVS[dq, :, :], zc[64:68, :].rearrange("p (b n) -> p b n", b=NB)
                )

            stages = [
                prefetch,
                stage0,
                lambda q: stage_h(q, 0),
                lambda q: stage_h(q, 1),
                lambda q: stage_h(q, 2),
                stage4,
            ]
            NS = len(stages)
            for t in range(NCHUNK + NS - 1):
                for s in range(NS - 1, -1, -1):
                    q = t - s
                    if 0 <= q < NCHUNK:
                        stages[s](q)

            # ---- stage D: phi assembly + reduction ----
            Q = sgpool.tile([P, NB, NSTEP], SD, tag="dQ")
            nc.vector.tensor_tensor(Q[:], U2S[:], TfS[:], ALU.mult)
            S = sgpool.tile([P, NB, NSTEP], SD, tag="dS")
            nc.vector.tensor_tensor(S[:], GVS[:], Q[:], ALU.subtract)
            E = sgpool.tile([P, NB, NSTEP], SD, tag="dE")
            nc.gpsimd.tensor_tensor(E[:], TfS[:], TfS[:], ALU.mult)
            sp = sgpool.tile([P, NB, NSTEP], SD, tag="dsp")
            nc.vector.tensor_scalar(sp[:], E[:], -0.25, 0.25, ALU.mult, ALU.add)
            S2 = sgpool.tile([P, NB, NSTEP], SD, tag="dS2")
            nc.vector.tensor_tensor(S2[:], S[:], sp[:], ALU.mult)
            vT = sgpool.tile([P, NB, 1], F32, tag="vT")
            nc.vector.tensor_reduce(vT[:], S2[:], mybir.AxisListType.X, ALU.add)

            # ---- outputs ----
            nc.sync.dma_start(yS_d[:], sfull[:, :, NSTEP : NSTEP + 1])
            nc.sync.dma_start(yV_d[:], vT[:])

    _legalize_waits(nc)
    return nc


def _prep_host(inputs):
    rnorm = np.ascontiguousarray(np.asarray(inputs["rnorm"], dtype=np.float32))
    W0 = np.asarray(inputs["W0"], dtype=np.float32)
    b0 = np.asarray(inputs["b0"], dtype=np.float32)
    Wh = np.asarray(inputs["Wh"], dtype=np.float32)
    bh = np.asarray(inputs["bh"], dtype=np.float32)
    Wf = np.asarray(inputs["Wf"], dtype=np.float32)
    bf = np.asarray(inputs["bf"], dtype=np.float32)

    sd_np = mybir.dt.np(SD)

    # static rhs rows: t-row (period 128), ones-row
    trow = np.ones((2, CC), np.float32)
    trow[0, :] = DT * np.tile(np.arange(NSTEP, dtype=np.float32), CC // NSTEP)

    # lhsT seeds [14, P]: row 3g+s for per-group streams, 12 t, 13 ones
    lhsT0 = np.zeros((14, P), np.float32)
    lhsTg = np.zeros((14, P), np.float32)
    lhsTu = np.zeros((14, P), np.float32)
    for g in range(NG):
        cols = slice(32 * g, 32 * (g + 1))
        lhsT0[3 * g + 0, cols] = W0[:, 1]              # s coefficient
        lhsTg[3 * g + 1, cols] = W0[:, 1]              # Ds row
        lhsTu[3 * g + 2, cols] = W0[:, 1] * np.sqrt(0.5) * SIG
        lhsT0[12, cols] = W0[:, 0]                     # t row
        lhsTg[13, cols] = W0[:, 0] * DT                # ones row -> dhdt*dt

    lhsTh = np.zeros((NH, P, P), np.float32)
    for l in range(NH):
        for g in range(NG):
            blk = slice(32 * g, 32 * (g + 1))
            lhsTh[l, blk, blk] = Wh[l].T
    lhsTf = np.zeros((P, NG), np.float32)
    for g in range(NG):
        lhsTf[32 * g : 32 * (g + 1), g] = Wf[0]

    bias = np.zeros((P, 4, 2), np.float32)
    bias[:, 0, 0] = np.tile(b0, NG)
    bias[:, 0, 1] = 0.5 * bias[:, 0, 0]
    for l in range(NH):
        bias[:, l + 1, 0] = np.tile(bh[l], NG)
        bias[:, l + 1, 1] = 0.5 * bias[:, l + 1, 0]
    bfh = np.full((P, 1), 0.5 * bf[0], np.float32)

    shared = {
        "trow": trow.astype(sd_np),
        "lhsT0": lhsT0.astype(sd_np),
        "lhsTg": lhsTg.astype(sd_np),
        "lhsTu": lhsTu.astype(sd_np),
        "lhsTh": lhsTh.astype(sd_np),
        "lhsTf": lhsTf.astype(sd_np),
        "bias": bias,
        "bfh": bfh,
    }

    in_maps = []
    for core in range(NCORE):
        shard = rnorm[core * BC : (core + 1) * BC]          # [1024, 128]
        sg = np.ascontiguousarray(
            shard.reshape(NB, P, NSTEP).transpose(1, 0, 2).reshape(P, NB * NSTEP)
        )
        in_maps.append({"rn_sg": sg, **shared})
    return in_maps


last_perf = {}


def kernel(trace=False, **inputs) -> np.ndarray:
    if "nc" not in _CACHE:
        _CACHE["nc"] = _build_program()
    nc = _CACHE["nc"]
    in_maps = _prep_host(inputs)
    res = run_bass_kernel_spmd(nc, in_maps, list(range(NCORE)), trace=trace)
    last_perf["exec_time_ns"] = res.exec_time_ns
    out = np.empty((B, 2), np.float32)
    for core in range(NCORE):
        yS = res.results[core]["yS"]                        # [128, 8]
        yV = res.results[core]["yV"]                        # [128, 8]
        blk = out[core * BC : (core + 1) * BC]
        blk[:, 0] = yS.T.reshape(-1)
        blk[:, 1] = yV.T.reshape(-1)
    return out
